# revision 1
# baseline (speedup 1.0000x reference)
"""Trainium2 Bass kernel for nn_Network_67388036874689.

Data-parallel over batch: B=256 sharded as 32 samples on each of 8 cores;
all parameters replicated (host-precomposed).

Structure exploited (validated against the reference on host):
  - fog_of_war's greedy scan returns arange(B) -> the permutation is identity.
  - conv2d(3x3, pad=1) on [C, H, 1] spatial input only sees kernel column 1
    -> 1D conv over H with 3 taps.
  - Embedding (V=14) + pair-maxpool + conv compose into per-tap tables
    CW[kh] = pairmax_table @ conv_w[:, :, kh].T, host-built. Device conv =
    one-hot(pair idx) matmuls against CW with +-1 column shifts over a
    per-sample zero-padded one-hot layout.
  - pair-max is symmetric, so the enemy table uses the canonical unordered
    pair index (105 rows -> single partition piece, 3 matmuls per psum
    block). The friend branch keeps the ordered 196-row table split
    112+84 (6 matmuls per block): computing the canonical index on device
    from the tokens hits a still-undiagnosed corruption of samples 30/31
    (even positions), so the friend index stays 14*even+odd which is one
    proven vector op.
  - Conv bias folds into the following linear's bias (host).
  - The manipulator conv input is constant over h -> the 8192x256 manip
    linear collapses to 3 reduced 64x256 matrices (host-summed over h).
  - floor(t) for the token discretization = round-to-nearest via the 2^23
    magic constant, then subtract is_ge correction. (The round-down
    variant with +2^23-0.5 is WRONG for t < 0.25: ulp below 2^23 is 0.5.)

All HBM loads are issued on the single sync HWDGE ring in consumption
order (x/idx row, enemy tables, elw stream, manip, friend tables, flw
stream): ring FIFO keeps small early loads ahead of the 2 MB weight
pieces; splitting across rings lets the SDMA engines' packet-granular
round-robin starve small-packet queues.

Precision: tables/linears in bf16 (host sim rel err ~1e-4, measured
3.1e-4 on hardware); manipulator path f32/f32r; psum accumulation f32.
"""

import numpy as np
import ml_dtypes
from contextlib import ExitStack

import concourse.bass as bass
import concourse.bacc as bacc
import concourse.mybir as mybir
import concourse.tile as tile
from concourse.masks import make_identity
from concourse.bass_utils import run_bass_kernel_spmd

F32 = mybir.dt.float32
F32R = mybir.dt.float32r
BF16 = mybir.dt.bfloat16
I32 = mybir.dt.int32
AF = mybir.ActivationFunctionType
ALU = mybir.AluOpType
AX = mybir.AxisListType

NCORES = 8
B = 256
BC = B // NCORES        # 32 samples per core
L = 256
V = 14
EMB = 512
H = L // 2              # 128 pooled positions
NPAIR = V * V           # 196
P0 = 112                # friend pair-table partition split: 112 + 84
P1 = NPAIR - P0
NSYM = V * (V + 1) // 2   # 105: enemy uses the symmetric (unordered) table
SW = H + 2              # 130: per-sample padded width in the one-hot tiles
OHW = BC * SW           # 4160
DEBUG_TAPS = False


def _dram_inputs(nc):
    t = {}

    def inp(name, shape, dt):
        t[name] = nc.dram_tensor(name, list(shape), dt, kind="ExternalInput").ap()

    inp("idxrowE", (1, BC * H), BF16)   # host: canonical sym pair idx, flat
    inp("cwE", (NSYM, 768), BF16)    # enemy symmetric CW table
    inp("cwF0", (P0, 768), BF16)     # friend CW tables, col = kh*256 + o
    inp("cwF1", (P1, 768), BF16)
    inp("elw3", (256, 128 * 128), BF16)   # [o, (h, j)]
    inp("fw2", (256, 128 * 14), BF16)   # host: (flw @ f2w) as [o, (h, j)]
    inp("mlwS", (64, 768), F32R)     # col = v*256 + j, v in (int, h0, hL)
    inp("wsumT", (128, 192), F32R)   # col = v*64 + o
    inp("mcb", (64,), F32)
    inp("elbe", (128,), F32)         # enemy lin bias + folded conv bias
    inp("mlb", (256,), F32)
    inp("f2b", (14,), F32)              # host: flbe @ f2w + f2b
    t["out"] = nc.dram_tensor("out", [BC, 14], F32, kind="ExternalOutput").ap()
    return t


def _tap(nc, io, name, ap):
    if not DEBUG_TAPS:
        return
    t = nc.dram_tensor("tap_" + name, list(ap.shape), ap.dtype,
                       kind="ExternalOutput").ap()
    io["tap_" + name] = t
    nc.gpsimd.dma_start(t, ap)


def build_kernel(nc, tc, ctx):
    io = _dram_inputs(nc)
    consts = ctx.enter_context(tc.tile_pool(name="consts", bufs=1))
    work = ctx.enter_context(tc.tile_pool(name="work", bufs=1))
    wpool = ctx.enter_context(tc.tile_pool(name="wstream", bufs=4))
    ohpool = ctx.enter_context(tc.tile_pool(name="ohpool", bufs=1))
    ppp = ctx.enter_context(tc.tile_pool(name="ppp", bufs=2, space="PSUM"))
    pconv = ctx.enter_context(tc.tile_pool(name="pconv", bufs=4, space="PSUM"))
    plin = ctx.enter_context(tc.tile_pool(name="plin", bufs=1, space="PSUM"))
    psm = ctx.enter_context(tc.tile_pool(name="psm", bufs=1, space="PSUM"))

    def ctile(shape, dt, tag):
        return consts.tile(shape, dt, tag=tag, name=tag)

    def wtile(shape, dt, tag):
        return work.tile(shape, dt, tag=tag, name=tag)

    # ---------------- constants & small weights ----------------
    identF = ctile([128, 128], F32, "identF")
    make_identity(nc, identF)
    iota_i = ctile([128, 1], I32, "iota_i")
    nc.gpsimd.iota(iota_i[:, :], pattern=[[0, 1]], base=0, channel_multiplier=1)
    iota_col = ctile([128, 1], F32, "iota_col")
    nc.vector.tensor_copy(iota_col[:, :], iota_i[:, :])
    ones_row = ctile([1, 128], BF16, "ones_row")
    nc.vector.memset(ones_row[:, :], 1.0)

    def bias_col(dram_vec, n, tag):
        col = ctile([n, 1], F32, tag)
        nc.gpsimd.dma_start(col[:, :], dram_vec)
        return col

    def bias_bcast(dram_vec, rows, width, tag):
        out = ctile([rows, width], F32, tag)
        nc.gpsimd.dma_start(out[:, :], dram_vec[None, :].partition_broadcast(rows))
        return out

    # enemy pair-index row first on the sync HWDGE ring (host-precomputed):
    # the whole front of the kernel needs it
    idxrowE = wtile([1, BC * H], BF16, "idxrowE")
    nc.sync.dma_start(idxrowE[:, :], io["idxrowE"])

    elbeB = bias_bcast(io["elbe"], BC, 128, "elbeB")
    mlbB = bias_bcast(io["mlb"], BC, 256, "mlbB")
    f2bB = bias_bcast(io["f2b"], BC, 14, "f2bB")
    mcb_col = bias_col(io["mcb"], 64, "mcb")

    def load(name, shape, dt):
        t = ctile(shape, dt, name)
        nc.sync.dma_start(t[:, :], io[name])
        return t

    # All HBM loads go on the single sync HWDGE ring in exact consumption
    # order: ring FIFO means the small early loads fully drain before the
    # big weight streams start. (Splitting across rings lets the SDMA
    # engines' packet-granular round-robin starve the small-packet queue:
    # 1.5 KB vs 16 KB packets -> the conv tables took 13+ us to land.)
    # elw3/flw3: [o(256), (h,j)]; piece = [o-half(128), 64 h x 128 j] = 2 MB
    def stream_weights(dram):
        pieces = []
        for half in range(2):
            for hb in range(2):
                p = wpool.tile([128, 64 * 128], BF16, tag="wp", name="wp")
                nc.sync.dma_start(
                    p[:, :], dram[half * 128:(half + 1) * 128,
                                  hb * 8192:(hb + 1) * 8192])
                pieces.append(p)
        return pieces

    cwE = load("cwE", [NSYM, 768], BF16)
    elwP = stream_weights(io["elw3"])
    wsumT = load("wsumT", [128, 192], F32R)
    mlwS = load("mlwS", [64, 768], F32R)
    cwF0 = load("cwF0", [P0, 768], BF16)
    cwF1 = load("cwF1", [P1, 768], BF16)
    fw2h = []
    for half in range(2):
        t = ctile([128, 128 * 14], BF16, f"fw2h{half}")
        nc.sync.dma_start(t[:, :], io["fw2"][half * 128:(half + 1) * 128, :])
        fw2h.append(t)

    # ---------------- stage helpers ----------------
    def build_oh_sym(idxrow, tag):
        """One-hot over the canonical (symmetric) pair-idx row."""
        oh = ohpool.tile([NSYM, OHW], BF16, tag="ohS", name=f"ohS{tag}")
        nc.vector.memset(oh[:, 0:OHW:SW], 0.0)
        nc.vector.memset(oh[:, SW - 1:OHW:SW], 0.0)
        for blk in range(8):
            pp = ppp.tile([NSYM, 512], F32, tag="pp", name="pp")
            nc.tensor.matmul(pp[:, :], ones_row[:, 0:NSYM],
                             idxrow[:, blk * 512:(blk + 1) * 512],
                             start=True, stop=True)
            dst = oh[:, blk * 4 * SW:(blk + 1) * 4 * SW] \
                .rearrange("p (s w) -> p s w", w=SW)[:, :, 1:129]
            nc.vector.tensor_scalar(dst, pp[:, :].rearrange(
                "p (s w) -> p s w", w=128), iota_col[0:NSYM, :], None,
                ALU.is_equal)
        return oh

    def conv_apply_sym(oh, cw, tag):
        acts = [wtile([128, BC * H], BF16, f"acts{tag}{oc}") for oc in range(2)]
        for oc in range(2):
            for blk in range(8):
                cp = pconv.tile([128, 512], F32, tag="cp", name="cp")
                for kh in range(3):
                    lhsT = cw[:, kh * 256 + oc * 128:
                              kh * 256 + (oc + 1) * 128]
                    rhs = oh[:, blk * 4 * SW:(blk + 1) * 4 * SW] \
                        .rearrange("p (s w) -> p s w", w=SW)[:, :, kh:kh + 128]
                    nc.tensor.matmul(cp[:, :], lhsT, rhs,
                                     start=(kh == 0), stop=(kh == 2))
                dst = acts[oc][:, blk * 512:(blk + 1) * 512]
                if blk % 2 == 0:
                    nc.scalar.activation(dst, cp[:, :], AF.Copy)
                else:
                    nc.vector.tensor_copy(dst, cp[:, :])
        return acts

    def build_oh(idxrow, tag):
        """One-hot over the pair-idx row [1, 4096] (col s*128+h), padded
        layout: col s*130 + 1 + h holds [idx[s,h] == t]; cols s*130 and
        s*130+129 are zero (conv boundary)."""
        oh0 = ohpool.tile([P0, OHW], BF16, tag="oh0", name=f"oh0{tag}")
        oh1 = ohpool.tile([P1, OHW], BF16, tag="oh1", name=f"oh1{tag}")
        for oh in (oh0, oh1):
            nc.vector.memset(oh[:, 0:OHW:SW], 0.0)
            nc.vector.memset(oh[:, SW - 1:OHW:SW], 0.0)
        for blk in range(8):
            pp = ppp.tile([P0, 512], F32, tag="pp", name="pp")
            nc.tensor.matmul(pp[:, :], ones_row[:, 0:P0],
                             idxrow[:, blk * 512:(blk + 1) * 512],
                             start=True, stop=True)
            src = pp[:, :].rearrange("p (s w) -> p s w", w=128)
            dst0 = oh0[:, blk * 4 * SW:(blk + 1) * 4 * SW] \
                .rearrange("p (s w) -> p s w", w=SW)[:, :, 1:129]
            nc.vector.tensor_scalar(dst0, src, iota_col[0:P0, :], None,
                                    ALU.is_equal)
            dst1 = oh1[:, blk * 4 * SW:(blk + 1) * 4 * SW] \
                .rearrange("p (s w) -> p s w", w=SW)[:, :, 1:129]
            nc.vector.tensor_scalar(dst1, src[0:P1], float(P0),
                                    iota_col[0:P1, :], ALU.subtract,
                                    ALU.is_equal)
        return oh0, oh1

    def conv_apply(oh0, oh1, cw0, cw1, tag):
        """y[o, (s,h)] = sum_kh CW_kh[idx[h+kh-1], o]; acts as 2 halves
        [128 o', 32*128 (s,h)] bf16."""
        acts = [wtile([128, BC * H], BF16, f"acts{tag}{oc}") for oc in range(2)]
        for oc in range(2):
            for blk in range(8):
                cp = pconv.tile([128, 512], F32, tag="cp", name="cp")
                n = 0
                for cw, oh in ((cw0, oh0), (cw1, oh1)):
                    for kh in range(3):
                        lhsT = cw[:, kh * 256 + oc * 128:
                                  kh * 256 + (oc + 1) * 128]
                        rhs = oh[:, blk * 4 * SW:(blk + 1) * 4 * SW] \
                            .rearrange("p (s w) -> p s w", w=SW)[:, :, kh:kh + 128]
                        nc.tensor.matmul(cp[:, :], lhsT, rhs,
                                         start=(n == 0), stop=(n == 5))
                        n += 1
                dst = acts[oc][:, blk * 512:(blk + 1) * 512]
                if blk % 2 == 0:
                    nc.scalar.activation(dst, cp[:, :], AF.Copy)
                else:
                    nc.vector.tensor_copy(dst, cp[:, :])
        return acts

    def big_linear(acts, pieces, tag):
        """lp[s, j] = sum_{o,h} acts[o][:, s*128+h] * W[(o,h), j]"""
        lp = plin.tile([BC, 128], F32, tag="lp", name=f"lp{tag}")
        for c in range(256):
            half, h = divmod(c, 128)
            piece = pieces[half * 2 + h // 64]
            lhsT = acts[half][:, h:h + (BC - 1) * 128 + 1:128]
            rhs = piece[:, (h % 64) * 128:(h % 64 + 1) * 128]
            nc.tensor.matmul(lp[:, :], lhsT, rhs,
                             start=(c == 0), stop=(c == 255))
        return lp

    # ---------------- enemy branch ----------------
    ohE = build_oh_sym(idxrowE, "E")
    actsE = conv_apply_sym(ohE, cwE, "E")
    _tap(nc, io, "actsE0", actsE[0][:, :])
    lpE = big_linear(actsE, elwP, "E")

    logitsE = wtile([BC, 128], F32, "logitsE")
    nc.vector.tensor_tensor(logitsE[:, :], lpE[:, :], elbeB[:, :], ALU.add)
    _tap(nc, io, "logitsE", logitsE[:, :])
    ExE = wtile([BC, 128], F32, "ExE")
    nc.scalar.activation(ExE[:, :], logitsE[:, :], AF.Exp)
    smE = wtile([BC, 1], F32, "smE")
    nc.vector.reduce_sum(smE[:, :], ExE[:, :], AX.X)
    rsE = wtile([BC, 1], F32, "rsE")
    nc.vector.reciprocal(rsE[:, :], smE[:, :])
    eout = wtile([BC, 128], F32, "eout")
    nc.vector.tensor_scalar(eout[:, :], ExE[:, :], rsE[:, :], None, ALU.mult)

    tpv = psm.tile([128, BC], F32, tag="sm", name="tpv")
    nc.tensor.transpose(tpv[:, :], eout[:, :], identF[0:BC, 0:BC])
    vT = wtile([128, BC], F32R, "vT")
    nc.vector.tensor_copy(vT[:, :], tpv[:, :])
    _tap(nc, io, "vT", vT[:, :])

    # ---------------- manipulator ----------------
    cxs = {}
    for i, v in enumerate(("int", "h0", "hL")):
        cx = psm.tile([64, BC], F32, tag="sm", name=f"cx{v}")
        nc.tensor.matmul(cx[:, :], wsumT[:, i * 64:(i + 1) * 64], vT[:, :],
                         start=True, stop=True)
        cxs[v] = wtile([64, BC], F32R, f"cxs_{v}")
        nc.scalar.activation(cxs[v][:, :], cx[:, :], AF.Relu, bias=mcb_col[:, :])
    mp = psm.tile([BC, 256], F32, tag="sm", name="mp")
    for i, v in enumerate(("int", "h0", "hL")):
        nc.tensor.matmul(mp[:, :], cxs[v][:, :], mlwS[:, i * 256:(i + 1) * 256],
                         start=(i == 0), stop=(i == 2))
    m_sb = wtile([BC, 256], F32, "m_sb")
    nc.vector.tensor_tensor(m_sb[:, :], mp[:, :], mlbB[:, :], ALU.add)
    _tap(nc, io, "m", m_sb[:, :])

    # tokens = floor(|m|*100) mod 14; pair idx = 14*even + odd
    # floor via the 2^23 magic-number trick; mod 14 via 2 conditional subtracts
    tt = wtile([BC, 256], F32, "tt")
    nc.scalar.activation(tt[:, :], m_sb[:, :], AF.Abs, scale=100.0)
    fr0 = wtile([BC, 256], F32, "fr0")
    nc.vector.tensor_scalar(fr0[:, :], tt[:, :], 8388608.0, 8388608.0,
                            ALU.add, ALU.subtract)
    ge = wtile([BC, 256], F32, "ge")
    nc.vector.tensor_tensor(ge[:, :], tt[:, :], fr0[:, :], ALU.is_ge)
    fr = wtile([BC, 256], F32, "fr")
    nc.vector.scalar_tensor_tensor(fr[:, :], ge[:, :], -1.0, fr0[:, :],
                                   ALU.add, ALU.add)
    ti = wtile([BC, 256], F32, "ti")
    nc.vector.tensor_scalar(ti[:, :], fr[:, :], float(V), None, ALU.is_ge)
    t1 = wtile([BC, 256], F32, "t1")
    nc.vector.scalar_tensor_tensor(t1[:, :], ti[:, :], -float(V), fr[:, :],
                                   ALU.mult, ALU.add)
    t2 = wtile([BC, 256], F32, "t2")
    nc.vector.tensor_scalar(t2[:, :], t1[:, :], float(V), None, ALU.is_ge)
    tok = wtile([BC, 256], F32, "tok")
    nc.vector.scalar_tensor_tensor(tok[:, :], t2[:, :], -float(V), t1[:, :],
                                   ALU.mult, ALU.add)
    _tap(nc, io, "tok", tok[:, :])
    idxF = wtile([BC, H], BF16, "idxF")
    nc.vector.scalar_tensor_tensor(idxF[:, :], tok[:, 0:256:2], float(V),
                                   tok[:, 1:256:2], ALU.mult, ALU.add)
    idxrowF = wtile([1, BC * H], BF16, "idxrowF")
    nc.sync.dma_start(idxrowF[:, :], idxF[:, :])

    # ---------------- friend branch ----------------
    ohF0, ohF1 = build_oh(idxrowF, "F")
    actsF = conv_apply(ohF0, ohF1, cwF0, cwF1, "F")
    lpF = plin.tile([BC, 14], F32, tag="lp", name="lpF")
    for c in range(256):
        half, h = divmod(c, 128)
        lhsT = actsF[half][:, h:h + (BC - 1) * 128 + 1:128]
        rhs = fw2h[half][:, h * 14:(h + 1) * 14]
        nc.tensor.matmul(lpF[:, :], lhsT, rhs,
                         start=(c == 0), stop=(c == 255))
    logits = wtile([BC, 14], F32, "logits")
    nc.vector.tensor_tensor(logits[:, :], lpF[:, :], f2bB[:, :], ALU.add)
    ex = wtile([BC, 14], F32, "ex")
    nc.scalar.activation(ex[:, :], logits[:, :], AF.Exp)
    sm = wtile([BC, 1], F32, "sm")
    nc.vector.reduce_sum(sm[:, :], ex[:, :], AX.X)
    rs = wtile([BC, 1], F32, "rs")
    nc.vector.reciprocal(rs[:, :], sm[:, :])
    outt = wtile([BC, 14], F32, "outt")
    nc.vector.tensor_scalar(outt[:, :], ex[:, :], rs[:, :], None, ALU.mult)
    nc.sync.dma_start(io["out"], outt[:, :])


_CACHE = {}


def _get_nc():
    if "nc" not in _CACHE:
        nc = bacc.Bacc("TRN2", target_bir_lowering=False, debug=False,
                       num_devices=NCORES)
        with tile.TileContext(nc) as tc:
            with ExitStack() as ctx:
                build_kernel(nc, tc, ctx)
        nc.compile()
        _CACHE["nc"] = nc
    return _CACHE["nc"]


def prep_inputs(inputs):
    """Host-side composition + shard. Returns list of 8 in_maps."""
    f32 = np.float32
    bf16 = ml_dtypes.bfloat16

    def cw_tables(emb, cw_full, t0, t1):
        emb = np.asarray(emb, f32)
        cw = np.ascontiguousarray(np.asarray(cw_full, f32)[:, :, :, 1])  # [O,I,3]
        table = np.maximum(emb[t0], emb[t1])
        return np.concatenate([table @ cw[:, :, kh].T for kh in range(3)],
                              axis=1).astype(bf16)

    los, his = zip(*[(lo, hi) for lo in range(V) for hi in range(lo, V)])
    los, his = np.array(los), np.array(his)
    cwE = np.ascontiguousarray(
        cw_tables(inputs["enemy_emb"], inputs["enemy_conv_w"], los, his))
    ta, tb = np.meshgrid(np.arange(V), np.arange(V), indexing="ij")
    cwF = cw_tables(inputs["friend_emb"], inputs["friend_conv_w"],
                    ta.ravel(), tb.ravel())
    cwF0 = np.ascontiguousarray(cwF[:P0])
    cwF1 = np.ascontiguousarray(cwF[P0:])

    elw = np.asarray(inputs["enemy_lin_w"], f32)
    flw = np.asarray(inputs["friend_lin1_w"], f32)
    f2w = np.asarray(inputs["friend_lin2_w"], f32)
    elbe = (np.asarray(inputs["enemy_lin_b"], f32)
            + np.einsum("o,ohj->j", np.asarray(inputs["enemy_conv_b"], f32),
                        elw.reshape(256, 128, 128), optimize=True)).astype(f32)
    flbe = (np.asarray(inputs["friend_lin1_b"], f32)
            + np.einsum("o,ohj->j", np.asarray(inputs["friend_conv_b"], f32),
                        flw.reshape(256, 128, 128), optimize=True))
    fw2 = np.ascontiguousarray((flw @ f2w).reshape(256, 128 * 14)).astype(bf16)
    f2be = (flbe @ f2w + np.asarray(inputs["friend_lin2_b"], f32)).astype(f32)

    mcw = np.asarray(inputs["manip_conv_w"], f32)[:, :, :, 1]  # [64,128,3]
    s_int = mcw.sum(2)
    s12 = mcw[:, :, 1] + mcw[:, :, 2]
    s01 = mcw[:, :, 0] + mcw[:, :, 1]
    wsumT = np.concatenate([s_int.T, s12.T, s01.T], axis=1).astype(f32)  # [128,192]

    mlw3 = np.asarray(inputs["manip_lin_w"], f32).reshape(64, 128, 256)
    mlwS = np.concatenate([mlw3[:, 1:127].sum(1), mlw3[:, 0], mlw3[:, 127]],
                          axis=1).astype(f32)                            # [64,768]

    common = {
        "cwE": cwE, "cwF0": cwF0, "cwF1": cwF1,
        "elw3": np.ascontiguousarray(elw.reshape(256, 128 * 128)).astype(bf16),
        "fw2": fw2,
        "mlwS": np.ascontiguousarray(mlwS),
        "wsumT": np.ascontiguousarray(wsumT),
        "mcb": np.ascontiguousarray(inputs["manip_conv_b"], f32),
        "elbe": elbe,
        "mlb": np.ascontiguousarray(inputs["manip_lin_b"], f32),
        "f2b": f2be,
    }
    x = np.asarray(inputs["x"], np.int64)
    xlo = np.minimum(x[:, 0::2], x[:, 1::2])
    xhi = np.maximum(x[:, 0::2], x[:, 1::2])
    idxrow = (xhi + xlo * (27 - xlo) // 2).astype(bf16)   # [B, 128], ints < 105
    return [dict(common,
                 idxrowE=np.ascontiguousarray(
                     idxrow[c * BC:(c + 1) * BC].reshape(1, BC * H)))
            for c in range(NCORES)]


def kernel(**inputs):
    nc = _get_nc()
    in_maps = prep_inputs(inputs)
    res = run_bass_kernel_spmd(nc, in_maps, core_ids=list(range(NCORES)))
    return np.concatenate([r["out"] for r in res.results], axis=0)



# revision 5
# speedup vs baseline: 1.9270x; 1.9270x over previous
"""Trainium2 Bass kernel for nn_Network_67388036874689.

Data-parallel over batch: B=256 as 32 samples on each of 8 cores; params
replicated (host-precomposed).

Structure exploited (validated against the reference on host):
  - fog_of_war's greedy scan returns arange(B) -> permutation is identity.
  - conv2d(3x3, pad=1) on [C, H, 1] only sees kernel column 1 -> 1D 3-tap
    conv over H.
  - THE BIG FUSION: embedding + pair-maxpool + conv + linear collapse into
    a single gathered table per branch:
        logits[b, j] = sum_h G[pid[b, h], h, j]
    with G[p, h, j] = sum_kh CW_kh[p, :] @ W[:, h+1-kh, j] (host-built,
    boundary-clipped), pid = canonical unordered pair index (105 rows,
    pair-max is symmetric).  Linear bias folds in as bias[j]/128 added to
    every G[p, h, j] (exactly one p fires per h).
  - On device each branch is: one-hot(pid) built 512 cols at a time
    (ones-matmul broadcast + is_equal vs iota), then 128 accumulating
    matmuls lpsum[32, J] += OH[:, h]^T @ G[:, h, :], 4-way col-tiled
    (tile_position=(0,32j), h mod 4) so 4 streams share the PE array.
  - Enemy table is fp8 e4m3 (scaled by sE, descale folded into the Exp
    activation scale); host sim rel err 3.8e-4 vs 2e-2 tolerance.  Friend
    table bf16.  One-hots match table dtype; psum accumulates f32.
  - Manipulator conv input is constant over h -> collapses to 3 reduced
    64x256 matrices (host-summed); f32/f32r path (token discretization is
    precision-sensitive).
  - floor via the 2^23 round-to-nearest magic + is_ge correction; mod 14
    via 2 conditional subtracts; canonical sym pair index from tokens via
    |e-o| (no min/max): idx = hi + lo*(27-lo)/2 = z2/2 + 6.75w - w^2/8
    with w = (e+o) - |e-o|, z2 = (e+o) + |e-o|.

DMA: big streams (idx row, GE pieces in consumption order, GF) on the
sync HWDGE ring; small manipulator weights/biases on the gpsimd ring.
"""

import numpy as np
import ml_dtypes
from contextlib import ExitStack

import concourse.bass as bass
import concourse.bacc as bacc
import concourse.mybir as mybir
import concourse.tile as tile
from concourse.masks import make_identity
from concourse.bass_utils import run_bass_kernel_spmd

F32 = mybir.dt.float32
F32R = mybir.dt.float32r
BF16 = mybir.dt.bfloat16
FP8 = mybir.dt.float8e4
I32 = mybir.dt.int32
AF = mybir.ActivationFunctionType
ALU = mybir.AluOpType
AX = mybir.AxisListType

NCORES = 8
B = 256
BC = B // NCORES        # 32 samples per core
L = 256
V = 14
H = L // 2              # 128 pooled positions
NSYM = V * (V + 1) // 2  # 105 canonical pairs
SE_SCALE = None          # set by prep_inputs; descale folded into Exp
DEBUG_TAPS = False


def _dram_inputs(nc):
    t = {}

    def inp(name, shape, dt):
        t[name] = nc.dram_tensor(name, list(shape), dt, kind="ExternalInput").ap()

    inp("idxrowE", (1, BC * H), BF16)    # h-major: col = h*32 + b
    inp("GE", (NSYM, H * 128), FP8)      # col = h*128 + j, pre-scaled by sE
    inp("GF", (NSYM, H * 14), BF16)      # col = h*14 + n, bias folded
    inp("wsumT", (128, 192), F32R)       # col = v*64 + o
    inp("mlwS", (64, 768), F32R)         # col = v*256 + j
    inp("mcb", (64,), F32)
    inp("mlb", (256,), F32)
    t["out"] = nc.dram_tensor("out", [BC, 14], F32, kind="ExternalOutput").ap()
    return t


def _tap(nc, io, name, ap):
    if not DEBUG_TAPS:
        return
    t = nc.dram_tensor("tap_" + name, list(ap.shape), ap.dtype,
                       kind="ExternalOutput").ap()
    io["tap_" + name] = t
    nc.gpsimd.dma_start(t, ap)


def build_kernel(nc, tc, ctx, inv_se):
    io = _dram_inputs(nc)
    consts = ctx.enter_context(tc.tile_pool(name="consts", bufs=1))
    work = ctx.enter_context(tc.tile_pool(name="work", bufs=1))
    gepool = ctx.enter_context(tc.tile_pool(name="gepool", bufs=8))
    ppp = ctx.enter_context(tc.tile_pool(name="ppp", bufs=2, space="PSUM"))
    pacc = ctx.enter_context(tc.tile_pool(name="pacc", bufs=1, space="PSUM"))
    psm = ctx.enter_context(tc.tile_pool(name="psm", bufs=1, space="PSUM"))

    def ctile(shape, dt, tag):
        return consts.tile(shape, dt, tag=tag, name=tag)

    def wtile(shape, dt, tag):
        return work.tile(shape, dt, tag=tag, name=tag)

    # ---------------- constants & small weights ----------------
    identF = ctile([128, 128], F32, "identF")
    make_identity(nc, identF)
    iota_i = ctile([128, 1], I32, "iota_i")
    nc.gpsimd.iota(iota_i[:, :], pattern=[[0, 1]], base=0, channel_multiplier=1)
    iota_col = ctile([128, 1], F32, "iota_col")
    nc.vector.tensor_copy(iota_col[:, :], iota_i[:, :])
    ones_row = ctile([1, 128], BF16, "ones_row")
    nc.vector.memset(ones_row[:, :], 1.0)

    # first on the sync ring: the whole front of the kernel needs it
    idxrowE = wtile([1, BC * H], BF16, "idxrowE")
    nc.sync.dma_start(idxrowE[:, :], io["idxrowE"])

    # GE streamed in 8 pieces in consumption (h) order on the sync ring
    GEp = []
    for k in range(8):
        p = gepool.tile([NSYM, 16 * 128], FP8, tag="gep", name=f"gep{k}")
        nc.sync.dma_start(p[:, :], io["GE"][:, k * 2048:(k + 1) * 2048])
        GEp.append(p)
    GF = ctile([NSYM, H * 14], BF16, "GF")
    nc.sync.dma_start(GF[:, :], io["GF"])

    # small manipulator weights on the gpsimd ring (won't starve: tiny)
    mcb_col = ctile([64, 1], F32, "mcb")
    nc.gpsimd.dma_start(mcb_col[:, :], io["mcb"])
    mlbB = ctile([BC, 256], F32, "mlbB")
    nc.gpsimd.dma_start(mlbB[:, :], io["mlb"][None, :].partition_broadcast(BC))
    wsumT = ctile([128, 192], F32R, "wsumT")
    nc.gpsimd.dma_start(wsumT[:, :], io["wsumT"])
    mlwS = ctile([64, 768], F32R, "mlwS")
    nc.gpsimd.dma_start(mlwS[:, :], io["mlwS"])

    # ---------------- gathered-table branch helper ----------------
    def branch(idxrow, Gslices, J, lptile, oh_dt, tag):
        """lptile[32j:32j+32, :J] += OH[:, h]^T @ G_h for h%4==j."""
        oh = wtile([NSYM, BC * H], oh_dt, f"oh{tag}")
        for k in range(8):
            pp = ppp.tile([NSYM, 512], F32, tag="pp", name="pp")
            nc.tensor.matmul(pp[:, :], ones_row[:, 0:NSYM],
                             idxrow[:, k * 512:(k + 1) * 512],
                             start=True, stop=True)
            nc.vector.tensor_scalar(oh[:, k * 512:(k + 1) * 512], pp[:, :],
                                    iota_col[0:NSYM, :], None, ALU.is_equal)
            for hh in range(16):
                h = k * 16 + hh
                j = h % 4
                nc.tensor.matmul(
                    lptile[32 * j:32 * (j + 1), 0:J],
                    oh[:, h * 32:(h + 1) * 32],
                    Gslices(k, hh),
                    start=(h < 4), stop=(h >= 124),
                    tile_position=(0, 32 * j), skip_group_check=True)

    # ---------------- enemy branch ----------------
    lpE = pacc.tile([128, 128], F32, tag="lpE", name="lpE")
    branch(idxrowE, lambda k, hh: GEp[k][:, hh * 128:(hh + 1) * 128],
           128, lpE, FP8, "E")
    # combine 4 col-tiles; descale (1/sE) folded into Exp's scale.
    # Only one PSUM operand per DVE op -> copy slices 1..3 to SBUF first
    # (32-partition cross-quadrant reads are HW-verified on DVE).
    cE1 = wtile([BC, 128], F32, "cE1")
    nc.vector.tensor_copy(cE1[:, :], lpE[32:64, :])
    cE2 = wtile([BC, 128], F32, "cE2")
    nc.vector.tensor_copy(cE2[:, :], lpE[64:96, :])
    cE3 = wtile([BC, 128], F32, "cE3")
    nc.vector.tensor_copy(cE3[:, :], lpE[96:128, :])
    a01 = wtile([BC, 128], F32, "a01")
    nc.vector.tensor_tensor(a01[:, :], lpE[0:32, :], cE1[:, :], ALU.add)
    a23 = wtile([BC, 128], F32, "a23")
    nc.vector.tensor_tensor(a23[:, :], cE2[:, :], cE3[:, :], ALU.add)
    logitsE = wtile([BC, 128], F32, "logitsE")
    nc.vector.tensor_tensor(logitsE[:, :], a01[:, :], a23[:, :], ALU.add)
    _tap(nc, io, "logitsE", logitsE[:, :])
    ExE = wtile([BC, 128], F32, "ExE")
    nc.scalar.activation(ExE[:, :], logitsE[:, :], AF.Exp, scale=float(inv_se))
    smE = wtile([BC, 1], F32, "smE")
    nc.vector.reduce_sum(smE[:, :], ExE[:, :], AX.X)
    rsE = wtile([BC, 1], F32, "rsE")
    nc.vector.reciprocal(rsE[:, :], smE[:, :])
    eout = wtile([BC, 128], F32, "eout")
    nc.vector.tensor_scalar(eout[:, :], ExE[:, :], rsE[:, :], None, ALU.mult)
    _tap(nc, io, "eout", eout[:, :])

    # ---------------- manipulator ----------------
    tpv = psm.tile([128, BC], F32, tag="sm", name="tpv")
    nc.tensor.transpose(tpv[:, :], eout[:, :], identF[0:BC, 0:BC])
    vT = wtile([128, BC], F32R, "vT")
    nc.vector.tensor_copy(vT[:, :], tpv[:, :])
    cxs = {}
    for i, v in enumerate(("int", "h0", "hL")):
        cx = psm.tile([64, BC], F32, tag="sm", name=f"cx{v}")
        nc.tensor.matmul(cx[:, :], wsumT[:, i * 64:(i + 1) * 64], vT[:, :],
                         start=True, stop=True)
        cxs[v] = wtile([64, BC], F32R, f"cxs_{v}")
        nc.scalar.activation(cxs[v][:, :], cx[:, :], AF.Relu, bias=mcb_col[:, :])
    mp = psm.tile([BC, 256], F32, tag="sm", name="mp")
    for i, v in enumerate(("int", "h0", "hL")):
        nc.tensor.matmul(mp[:, :], cxs[v][:, :], mlwS[:, i * 256:(i + 1) * 256],
                         start=(i == 0), stop=(i == 2))
    m_sb = wtile([BC, 256], F32, "m_sb")
    nc.vector.tensor_tensor(m_sb[:, :], mp[:, :], mlbB[:, :], ALU.add)
    _tap(nc, io, "m", m_sb[:, :])

    # tokens = floor(|m|*100) mod 14 (floor: 2^23 round + is_ge correction)
    tt = wtile([BC, 256], F32, "tt")
    nc.scalar.activation(tt[:, :], m_sb[:, :], AF.Abs, scale=100.0)
    fr0 = wtile([BC, 256], F32, "fr0")
    nc.vector.tensor_scalar(fr0[:, :], tt[:, :], 8388608.0, 8388608.0,
                            ALU.add, ALU.subtract)
    ge = wtile([BC, 256], F32, "ge")
    nc.vector.tensor_tensor(ge[:, :], tt[:, :], fr0[:, :], ALU.is_ge)
    fr = wtile([BC, 256], F32, "fr")
    nc.vector.scalar_tensor_tensor(fr[:, :], ge[:, :], -1.0, fr0[:, :],
                                   ALU.add, ALU.add)
    ti = wtile([BC, 256], F32, "ti")
    nc.vector.tensor_scalar(ti[:, :], fr[:, :], float(V), None, ALU.is_ge)
    t1 = wtile([BC, 256], F32, "t1")
    nc.vector.scalar_tensor_tensor(t1[:, :], ti[:, :], -float(V), fr[:, :],
                                   ALU.mult, ALU.add)
    t2 = wtile([BC, 256], F32, "t2")
    nc.vector.tensor_scalar(t2[:, :], t1[:, :], float(V), None, ALU.is_ge)
    tok = wtile([BC, 256], F32, "tok")
    nc.vector.scalar_tensor_tensor(tok[:, :], t2[:, :], -float(V), t1[:, :],
                                   ALU.mult, ALU.add)
    _tap(nc, io, "tok", tok[:, :])

    # canonical sym pair index: w = (e+o)-|e-o| (=2lo), z2 = (e+o)+|e-o| (=2hi)
    # idx = z2/2 + 6.75*w - w*w/8
    e_, o_ = tok[:, 0:256:2], tok[:, 1:256:2]
    s_ = wtile([BC, H], F32, "s_")
    nc.vector.tensor_tensor(s_[:, :], e_, o_, ALU.add)
    d_ = wtile([BC, H], F32, "d_")
    nc.vector.tensor_tensor(d_[:, :], e_, o_, ALU.subtract)
    a_ = wtile([BC, H], F32, "a_")
    nc.scalar.activation(a_[:, :], d_[:, :], AF.Abs)
    w_ = wtile([BC, H], F32, "w_")
    nc.vector.tensor_tensor(w_[:, :], s_[:, :], a_[:, :], ALU.subtract)
    # z2h = (s+a)*0.5: halve a on the scalar engine, then (s*0.5) + a*0.5
    ah = wtile([BC, H], F32, "ah")
    nc.scalar.activation(ah[:, :], a_[:, :], AF.Copy, scale=0.5)
    z2h = wtile([BC, H], F32, "z2h")
    nc.vector.scalar_tensor_tensor(z2h[:, :], s_[:, :], 0.5, ah[:, :],
                                   ALU.mult, ALU.add)
    q_ = wtile([BC, H], F32, "q_")
    nc.vector.tensor_tensor(q_[:, :], w_[:, :], w_[:, :], ALU.mult)
    u1 = wtile([BC, H], F32, "u1")
    nc.vector.scalar_tensor_tensor(u1[:, :], w_[:, :], 6.75, z2h[:, :],
                                   ALU.mult, ALU.add)
    idxF = wtile([BC, H], F32, "idxF")
    nc.vector.scalar_tensor_tensor(idxF[:, :], q_[:, :], -0.125, u1[:, :],
                                   ALU.mult, ALU.add)
    _tap(nc, io, "idxF", idxF[:, :])

    # transpose to h-major and flatten to a single row
    tpF = psm.tile([128, BC], F32, tag="sm", name="tpF")
    nc.tensor.transpose(tpF[:, :], idxF[:, :], identF[0:BC, 0:BC])
    idxFT = wtile([128, BC], BF16, "idxFT")
    nc.vector.tensor_copy(idxFT[:, :], tpF[:, :])
    idxrowF = wtile([1, BC * H], BF16, "idxrowF")
    nc.sync.dma_start(idxrowF[:, :], idxFT[:, :])

    # ---------------- friend branch ----------------
    lpF = pacc.tile([128, 14], F32, tag="lpF", name="lpF")
    branch(idxrowF, lambda k, hh: GF[:, (k * 16 + hh) * 14:(k * 16 + hh + 1) * 14],
           14, lpF, BF16, "F")
    cF1 = wtile([BC, 14], F32, "cF1")
    nc.vector.tensor_copy(cF1[:, :], lpF[32:64, 0:14])
    cF2 = wtile([BC, 14], F32, "cF2")
    nc.vector.tensor_copy(cF2[:, :], lpF[64:96, 0:14])
    cF3 = wtile([BC, 14], F32, "cF3")
    nc.vector.tensor_copy(cF3[:, :], lpF[96:128, 0:14])
    b01 = wtile([BC, 14], F32, "b01")
    nc.vector.tensor_tensor(b01[:, :], lpF[0:32, 0:14], cF1[:, :], ALU.add)
    b23 = wtile([BC, 14], F32, "b23")
    nc.vector.tensor_tensor(b23[:, :], cF2[:, :], cF3[:, :], ALU.add)
    logitsF = wtile([BC, 14], F32, "logitsF")
    nc.vector.tensor_tensor(logitsF[:, :], b01[:, :], b23[:, :], ALU.add)
    ex = wtile([BC, 14], F32, "ex")
    nc.scalar.activation(ex[:, :], logitsF[:, :], AF.Exp)
    sm = wtile([BC, 1], F32, "sm")
    nc.vector.reduce_sum(sm[:, :], ex[:, :], AX.X)
    rs = wtile([BC, 1], F32, "rs")
    nc.vector.reciprocal(rs[:, :], sm[:, :])
    outt = wtile([BC, 14], F32, "outt")
    nc.vector.tensor_scalar(outt[:, :], ex[:, :], rs[:, :], None, ALU.mult)
    nc.sync.dma_start(io["out"], outt[:, :])


_CACHE = {}


def _get_nc(inv_se):
    key = ("nc", round(float(inv_se), 10))
    if key not in _CACHE:
        nc = bacc.Bacc("TRN2", target_bir_lowering=False, debug=False,
                       num_devices=NCORES)
        with tile.TileContext(nc) as tc:
            with ExitStack() as ctx:
                build_kernel(nc, tc, ctx, inv_se)
        nc.compile()
        _CACHE[key] = nc
    return _CACHE[key]


def _build_G(emb, conv_w, lin_w, t0, t1, out_w=None):
    """G[p, h, j]: logits[b, j] = sum_h G[pid[b, h], h, j]."""
    f32 = np.float32
    emb = np.asarray(emb, f32)
    cw = np.ascontiguousarray(np.asarray(conv_w, f32)[:, :, :, 1])  # [O,I,3]
    table = np.maximum(emb[t0], emb[t1])                            # [P,512]
    CW = [table @ cw[:, :, kh].T for kh in range(3)]                # [P,256]
    W = np.asarray(lin_w, f32).reshape(256, H, -1)                  # [O,H,J]
    if out_w is not None:
        W = np.einsum("ohj,jn->ohn", W, np.asarray(out_w, f32))
    G = np.einsum("po,ohj->phj", CW[1], W)
    G[:, 0:H - 1] += np.einsum("po,ohj->phj", CW[0], W[:, 1:H])
    G[:, 1:H] += np.einsum("po,ohj->phj", CW[2], W[:, 0:H - 1])
    return G


def prep_inputs(inputs):
    """Host-side composition + shard. Returns (list of 8 in_maps, inv_se)."""
    f32 = np.float32
    bf16 = ml_dtypes.bfloat16
    fp8 = ml_dtypes.float8_e4m3fn

    los, his = zip(*[(lo, hi) for lo in range(V) for hi in range(lo, V)])
    los, his = np.array(los), np.array(his)

    elw3 = np.asarray(inputs["enemy_lin_w"], f32).reshape(256, H, 128)
    elbe = (np.asarray(inputs["enemy_lin_b"], f32)
            + np.einsum("o,ohj->j", np.asarray(inputs["enemy_conv_b"], f32),
                        elw3, optimize=True))
    GE = _build_G(inputs["enemy_emb"], inputs["enemy_conv_w"],
                  inputs["enemy_lin_w"], los, his)
    GE += elbe[None, None, :] / H
    se = 240.0 / float(np.abs(GE).max())
    GEq = np.ascontiguousarray((GE * se).reshape(NSYM, H * 128)).astype(fp8)

    flw3 = np.asarray(inputs["friend_lin1_w"], f32).reshape(256, H, 128)
    f2w = np.asarray(inputs["friend_lin2_w"], f32)
    flbe = (np.asarray(inputs["friend_lin1_b"], f32)
            + np.einsum("o,ohj->j", np.asarray(inputs["friend_conv_b"], f32),
                        flw3, optimize=True))
    f2be = flbe @ f2w + np.asarray(inputs["friend_lin2_b"], f32)
    GF = _build_G(inputs["friend_emb"], inputs["friend_conv_w"],
                  inputs["friend_lin1_w"], los, his, out_w=f2w)
    GF += f2be[None, None, :] / H
    GFq = np.ascontiguousarray(GF.reshape(NSYM, H * 14)).astype(bf16)

    mcw = np.asarray(inputs["manip_conv_w"], f32)[:, :, :, 1]  # [64,128,3]
    s_int = mcw.sum(2)
    s12 = mcw[:, :, 1] + mcw[:, :, 2]
    s01 = mcw[:, :, 0] + mcw[:, :, 1]
    wsumT = np.concatenate([s_int.T, s12.T, s01.T], axis=1).astype(f32)
    mlw3 = np.asarray(inputs["manip_lin_w"], f32).reshape(64, 128, 256)
    mlwS = np.concatenate([mlw3[:, 1:127].sum(1), mlw3[:, 0], mlw3[:, 127]],
                          axis=1).astype(f32)

    common = {
        "GE": GEq, "GF": GFq,
        "wsumT": np.ascontiguousarray(wsumT),
        "mlwS": np.ascontiguousarray(mlwS),
        "mcb": np.ascontiguousarray(inputs["manip_conv_b"], f32),
        "mlb": np.ascontiguousarray(inputs["manip_lin_b"], f32),
    }
    x = np.asarray(inputs["x"], np.int64)
    xlo = np.minimum(x[:, 0::2], x[:, 1::2])
    xhi = np.maximum(x[:, 0::2], x[:, 1::2])
    pid = (xhi + xlo * (27 - xlo) // 2).astype(bf16)   # [B, 128] ints < 105
    maps = []
    for c in range(NCORES):
        rowE = np.ascontiguousarray(
            pid[c * BC:(c + 1) * BC].T.reshape(1, BC * H))  # h-major
        maps.append(dict(common, idxrowE=rowE))
    return maps, 1.0 / se


def kernel(**inputs):
    in_maps, inv_se = prep_inputs(inputs)
    nc = _get_nc(inv_se)
    res = run_bass_kernel_spmd(nc, in_maps, core_ids=list(range(NCORES)))
    return np.concatenate([r["out"] for r in res.results], axis=0)


# revision 9
# speedup vs baseline: 2.0298x; 1.0534x over previous
"""Trainium2 Bass kernel for nn_Network_67388036874689.

Data-parallel over batch: B=256 as 32 samples on each of 8 cores; params
replicated (host-precomposed).

Structure exploited (validated against the reference on host):
  - fog_of_war's greedy scan returns arange(B) -> permutation is identity.
  - conv2d(3x3, pad=1) on [C, H, 1] only sees kernel column 1 -> 1D 3-tap
    conv over H.
  - THE BIG FUSION: embedding + pair-maxpool + conv + linear collapse into
    a single gathered table per branch:
        logits[b, j] = sum_h G[pid[b, h], h, j]
    with G[p, h, j] = sum_kh CW_kh[p, :] @ W[:, h+1-kh, j] (host-built,
    boundary-clipped), pid = canonical unordered pair index (105 rows,
    pair-max is symmetric).  Linear bias folds in as bias[j]/128 added to
    every G[p, h, j] (exactly one p fires per h).
  - On device each branch is 128 accumulating matmuls
    lpsum[32, J] += OH[:, h]^T @ G[:, h, :], 2-way col-tiled
    (tile_position=(0,32*(h%2))) so two streams share the PE array and the
    final combine is one copy + one add.
  - The ENEMY one-hot depends only on the host-known input x, so it is
    built on the host ([105, 4096] fp8) and streamed interleaved with the
    GE pieces in consumption order -> the enemy phase is pure DMA-paced
    gather with zero DVE work.
  - Enemy table fp8 e4m3 (scaled by sE; descale folded into Exp's scale);
    host sim rel err 3.8e-4 vs 2e-2 tolerance.  Friend table bf16.
  - Manipulator conv input is constant over h -> collapses to 3 reduced
    64x256 matrices (host-summed); f32/f32r path (token discretization is
    precision-sensitive).
  - floor via the 2^23 round-to-nearest magic + is_ge correction; mod 14
    via 2 conditional subtracts; friend pair index = canonical sym index
    from ALU min/max: idx = hi + lo*(27-lo)/2.
  - Friend one-hot: tokens are device-data, so the index row is
    transposed (PE), flattened (SBUF->SBUF DMA), partition-broadcast to
    105 rows per 512-col block (DMA, keeps PE/DVE free), then one
    bf16 is_equal per block.

DMA: sync HWDGE ring carries OHE_k/GE_k interleaved, GF, the idx-row
flatten + broadcasts, and the output; small manipulator weights on the
gpsimd SWDGE ring.
"""

import numpy as np
import ml_dtypes
from contextlib import ExitStack

import concourse.bass as bass
import concourse.bacc as bacc
import concourse.mybir as mybir
import concourse.tile as tile
from concourse.masks import make_identity
from concourse.bass_utils import run_bass_kernel_spmd

F32 = mybir.dt.float32
F32R = mybir.dt.float32r
BF16 = mybir.dt.bfloat16
FP8 = mybir.dt.float8e4
I32 = mybir.dt.int32
AF = mybir.ActivationFunctionType
ALU = mybir.AluOpType
AX = mybir.AxisListType

NCORES = 8
B = 256
BC = B // NCORES        # 32 samples per core
L = 256
V = 14
H = L // 2              # 128 pooled positions
NSYM = V * (V + 1) // 2  # 105 canonical pairs
DEBUG_TAPS = False


def _dram_inputs(nc):
    t = {}

    def inp(name, shape, dt):
        t[name] = nc.dram_tensor(name, list(shape), dt, kind="ExternalInput").ap()

    inp("OHE", (NSYM, BC * H), FP8)      # host one-hot, col = h*32 + b
    inp("GE", (NSYM, H * 128), FP8)      # col = h*128 + j, pre-scaled by sE
    inp("GF", (NSYM, H * 14), BF16)      # col = h*14 + n, bias folded
    inp("wsumT", (128, 192), F32R)       # col = v*64 + o
    inp("mlwS", (64, 768), F32R)         # col = v*256 + j
    inp("mcb", (64,), F32)
    inp("mlb", (256,), F32)
    t["out"] = nc.dram_tensor("out", [BC, 14], F32, kind="ExternalOutput").ap()
    return t


def _tap(nc, io, name, ap):
    if not DEBUG_TAPS:
        return
    t = nc.dram_tensor("tap_" + name, list(ap.shape), ap.dtype,
                       kind="ExternalOutput").ap()
    io["tap_" + name] = t
    nc.gpsimd.dma_start(t, ap)


def build_kernel(nc, tc, ctx, inv_se):
    io = _dram_inputs(nc)
    consts = ctx.enter_context(tc.tile_pool(name="consts", bufs=1))
    work = ctx.enter_context(tc.tile_pool(name="work", bufs=1))
    gepool = ctx.enter_context(tc.tile_pool(name="gepool", bufs=8))
    ohepool = ctx.enter_context(tc.tile_pool(name="ohepool", bufs=8))
    ohfpool = ctx.enter_context(tc.tile_pool(name="ohfpool", bufs=8))
    ppp = ctx.enter_context(tc.tile_pool(name="ppp", bufs=2, space="PSUM"))
    pacc = ctx.enter_context(tc.tile_pool(name="pacc", bufs=1, space="PSUM"))
    psm = ctx.enter_context(tc.tile_pool(name="psm", bufs=1, space="PSUM"))
    cxp = ctx.enter_context(tc.tile_pool(name="cxp", bufs=2, space="PSUM"))

    def ctile(shape, dt, tag):
        return consts.tile(shape, dt, tag=tag, name=tag)

    def wtile(shape, dt, tag):
        return work.tile(shape, dt, tag=tag, name=tag)

    # ---------------- constants & small weights ----------------
    identF = ctile([128, 128], F32, "identF")
    make_identity(nc, identF)
    iota_i = ctile([128, 1], I32, "iota_i")
    nc.gpsimd.iota(iota_i[:, :], pattern=[[0, 1]], base=0, channel_multiplier=1)
    iota_col = ctile([128, 1], F32, "iota_col")
    nc.vector.tensor_copy(iota_col[:, :], iota_i[:, :])

    # sync ring: OHE_k / GE_k interleaved in consumption order, then GF
    ohE, GEp = [], []
    for k in range(8):
        o = ohepool.tile([NSYM, 512], FP8, tag="ohe", name=f"ohe{k}")
        nc.sync.dma_start(o[:, :], io["OHE"][:, k * 512:(k + 1) * 512])
        ohE.append(o)
        g = gepool.tile([NSYM, 16 * 128], FP8, tag="gep", name=f"gep{k}")
        nc.sync.dma_start(g[:, :], io["GE"][:, k * 2048:(k + 1) * 2048])
        GEp.append(g)
    GF = ctile([NSYM, H * 14], BF16, "GF")
    nc.sync.dma_start(GF[:, :], io["GF"])

    # small manipulator weights on the gpsimd ring
    mcb_col = ctile([64, 1], F32, "mcb")
    nc.gpsimd.dma_start(mcb_col[:, :], io["mcb"])
    mlbB = ctile([BC, 256], F32, "mlbB")
    nc.gpsimd.dma_start(mlbB[:, :], io["mlb"][None, :].partition_broadcast(BC))
    wsumT = ctile([128, 192], F32R, "wsumT")
    nc.gpsimd.dma_start(wsumT[:, :], io["wsumT"])
    mlwS = ctile([64, 768], F32R, "mlwS")
    nc.gpsimd.dma_start(mlwS[:, :], io["mlwS"])

    # ---------------- enemy branch: pure DMA-paced gather ----------------
    lpE = pacc.tile([64, 128], F32, tag="lp", name="lpE")
    for k in range(8):
        for hh in range(16):
            h = k * 16 + hh
            j = h % 2
            nc.tensor.matmul(
                lpE[32 * j:32 * (j + 1), :],
                ohE[k][:, hh * 32:(hh + 1) * 32],
                GEp[k][:, hh * 128:(hh + 1) * 128],
                start=(h < 2), stop=(h >= 126),
                tile_position=(0, 32 * j), skip_group_check=True)

    cE1 = wtile([BC, 128], F32, "cE1")
    nc.vector.tensor_copy(cE1[:, :], lpE[32:64, :])
    logitsE = wtile([BC, 128], F32, "logitsE")
    nc.vector.tensor_tensor(logitsE[:, :], lpE[0:32, :], cE1[:, :], ALU.add)
    _tap(nc, io, "logitsE", logitsE[:, :])
    # softmax (descale 1/sE folded into Exp scale; sum fused via accum_out)
    ExE = wtile([BC, 128], F32, "ExE")
    smE = wtile([BC, 1], F32, "smE")
    nc.scalar.activation(ExE[:, :], logitsE[:, :], AF.Exp, scale=float(inv_se),
                         accum_out=smE[:, :])
    rsE = wtile([BC, 1], F32, "rsE")
    nc.vector.reciprocal(rsE[:, :], smE[:, :])
    eout = wtile([BC, 128], F32, "eout")
    nc.vector.tensor_scalar(eout[:, :], ExE[:, :], rsE[:, :], None, ALU.mult)
    _tap(nc, io, "eout", eout[:, :])

    # ---------------- manipulator ----------------
    tpv = psm.tile([128, BC], F32, tag="tp", name="tpv")
    nc.tensor.transpose(tpv[:, :], eout[:, :], identF[0:BC, 0:BC])
    vT = wtile([128, BC], F32R, "vT")
    nc.vector.tensor_copy(vT[:, :], tpv[:, :])
    cxs = {}
    for i, v in enumerate(("int", "h0", "hL")):
        cx = cxp.tile([64, BC], F32, tag="cx", name=f"cx{v}")
        nc.tensor.matmul(cx[:, :], wsumT[:, i * 64:(i + 1) * 64], vT[:, :],
                         start=True, stop=True)
        cxs[v] = wtile([64, BC], F32R, f"cxs_{v}")
        nc.scalar.activation(cxs[v][:, :], cx[:, :], AF.Relu, bias=mcb_col[:, :])
    mp = psm.tile([BC, 256], F32, tag="mp", name="mp")
    for i, v in enumerate(("int", "h0", "hL")):
        nc.tensor.matmul(mp[:, :], cxs[v][:, :], mlwS[:, i * 256:(i + 1) * 256],
                         start=(i == 0), stop=(i == 2))
    m_sb = wtile([BC, 256], F32, "m_sb")
    nc.vector.tensor_tensor(m_sb[:, :], mp[:, :], mlbB[:, :], ALU.add)
    _tap(nc, io, "m", m_sb[:, :])

    # tokens = floor(|m|*100) mod 14 (floor: 2^23 round + is_ge correction)
    tt = wtile([BC, 256], F32, "tt")
    nc.scalar.activation(tt[:, :], m_sb[:, :], AF.Abs, scale=100.0)
    fr0 = wtile([BC, 256], F32, "fr0")
    nc.vector.tensor_scalar(fr0[:, :], tt[:, :], 8388608.0, 8388608.0,
                            ALU.add, ALU.subtract)
    ge = wtile([BC, 256], F32, "ge")
    nc.vector.tensor_tensor(ge[:, :], tt[:, :], fr0[:, :], ALU.is_ge)
    fr = wtile([BC, 256], F32, "fr")
    nc.vector.scalar_tensor_tensor(fr[:, :], ge[:, :], -1.0, fr0[:, :],
                                   ALU.add, ALU.add)
    ti = wtile([BC, 256], F32, "ti")
    nc.vector.tensor_scalar(ti[:, :], fr[:, :], float(V), None, ALU.is_ge)
    t1 = wtile([BC, 256], F32, "t1")
    nc.vector.scalar_tensor_tensor(t1[:, :], ti[:, :], -float(V), fr[:, :],
                                   ALU.mult, ALU.add)
    t2 = wtile([BC, 256], F32, "t2")
    nc.vector.tensor_scalar(t2[:, :], t1[:, :], float(V), None, ALU.is_ge)
    tok = wtile([BC, 256], F32, "tok")
    nc.vector.scalar_tensor_tensor(tok[:, :], t2[:, :], -float(V), t1[:, :],
                                   ALU.mult, ALU.add)
    _tap(nc, io, "tok", tok[:, :])

    # canonical sym pair index via ALU min/max:
    # idx = hi + lo*(27-lo)/2 = hi + 13.5*lo - 0.5*lo^2
    e_, o_ = tok[:, 0:256:2], tok[:, 1:256:2]
    lo_ = wtile([BC, H], F32, "lo_")
    nc.vector.tensor_tensor(lo_[:, :], e_, o_, ALU.min)
    hi_ = wtile([BC, H], F32, "hi_")
    nc.vector.tensor_tensor(hi_[:, :], e_, o_, ALU.max)
    q_ = wtile([BC, H], F32, "q_")
    nc.vector.tensor_tensor(q_[:, :], lo_[:, :], lo_[:, :], ALU.mult)
    u1 = wtile([BC, H], F32, "u1")
    nc.vector.scalar_tensor_tensor(u1[:, :], lo_[:, :], 13.5, hi_[:, :],
                                   ALU.mult, ALU.add)
    idxF = wtile([BC, H], F32, "idxF")
    nc.vector.scalar_tensor_tensor(idxF[:, :], q_[:, :], -0.5, u1[:, :],
                                   ALU.mult, ALU.add)
    _tap(nc, io, "idxF", idxF[:, :])

    # transpose to h-major, flatten to a single row (2 chunk DMAs)
    tpF = psm.tile([128, BC], F32, tag="tp", name="tpF")
    nc.tensor.transpose(tpF[:, :], idxF[:, :], identF[0:BC, 0:BC])
    idxFT = wtile([128, BC], BF16, "idxFT")
    nc.vector.tensor_copy(idxFT[:, :], tpF[:, :])
    idxrowF = wtile([1, BC * H], BF16, "idxrowF")
    for c in range(2):
        nc.sync.dma_start(idxrowF[:, c * 2048:(c + 1) * 2048],
                          idxFT[c * 64:(c + 1) * 64, :])

    # ---------------- friend branch ----------------
    ones_row = ctile([1, NSYM], BF16, "ones_row")
    nc.vector.memset(ones_row[:, :], 1.0)
    lpF = pacc.tile([64, 14], F32, tag="lp", name="lpF")
    for k in range(8):
        pp = ppp.tile([NSYM, 512], F32, tag="pp", name="pp")
        nc.tensor.matmul(pp[:, :], ones_row[:, :],
                         idxrowF[:, k * 512:(k + 1) * 512],
                         start=True, stop=True)
        ohf = ohfpool.tile([NSYM, 512], BF16, tag="ohf", name=f"ohf{k}")
        nc.vector.tensor_scalar(ohf[:, :], pp[:, :], iota_col[0:NSYM, :],
                                None, ALU.is_equal)
        for hh in range(16):
            h = k * 16 + hh
            j = h % 2
            nc.tensor.matmul(
                lpF[32 * j:32 * (j + 1), 0:14],
                ohf[:, hh * 32:(hh + 1) * 32],
                GF[:, h * 14:(h + 1) * 14],
                start=(h < 2), stop=(h >= 126),
                tile_position=(0, 32 * j), skip_group_check=True)

    cF1 = wtile([BC, 14], F32, "cF1")
    nc.vector.tensor_copy(cF1[:, :], lpF[32:64, 0:14])
    logitsF = wtile([BC, 14], F32, "logitsF")
    nc.vector.tensor_tensor(logitsF[:, :], lpF[0:32, 0:14], cF1[:, :], ALU.add)
    ex = wtile([BC, 14], F32, "ex")
    sm = wtile([BC, 1], F32, "sm")
    nc.scalar.activation(ex[:, :], logitsF[:, :], AF.Exp, accum_out=sm[:, :])
    rs = wtile([BC, 1], F32, "rs")
    nc.vector.reciprocal(rs[:, :], sm[:, :])
    outt = wtile([BC, 14], F32, "outt")
    nc.vector.tensor_scalar(outt[:, :], ex[:, :], rs[:, :], None, ALU.mult)
    nc.sync.dma_start(io["out"], outt[:, :])


_CACHE = {}


def _get_nc(inv_se):
    key = ("nc", round(float(inv_se), 10))
    if key not in _CACHE:
        nc = bacc.Bacc("TRN2", target_bir_lowering=False, debug=False,
                       num_devices=NCORES)
        with tile.TileContext(nc) as tc:
            with ExitStack() as ctx:
                build_kernel(nc, tc, ctx, inv_se)
        nc.compile()
        _CACHE[key] = nc
    return _CACHE[key]


def _build_G(emb, conv_w, lin_w, t0, t1, out_w=None):
    """G[p, h, j]: logits[b, j] = sum_h G[pid[b, h], h, j]."""
    f32 = np.float32
    emb = np.asarray(emb, f32)
    cw = np.ascontiguousarray(np.asarray(conv_w, f32)[:, :, :, 1])  # [O,I,3]
    table = np.maximum(emb[t0], emb[t1])                            # [P,512]
    CW = [table @ cw[:, :, kh].T for kh in range(3)]                # [P,256]
    W = np.asarray(lin_w, f32).reshape(256, H, -1)                  # [O,H,J]
    if out_w is not None:
        W = np.einsum("ohj,jn->ohn", W, np.asarray(out_w, f32))
    G = np.einsum("po,ohj->phj", CW[1], W)
    G[:, 0:H - 1] += np.einsum("po,ohj->phj", CW[0], W[:, 1:H])
    G[:, 1:H] += np.einsum("po,ohj->phj", CW[2], W[:, 0:H - 1])
    return G


def prep_inputs(inputs):
    """Host-side composition + shard. Returns (list of 8 in_maps, inv_se)."""
    f32 = np.float32
    bf16 = ml_dtypes.bfloat16
    fp8 = ml_dtypes.float8_e4m3fn

    los, his = zip(*[(lo, hi) for lo in range(V) for hi in range(lo, V)])
    los, his = np.array(los), np.array(his)

    elw3 = np.asarray(inputs["enemy_lin_w"], f32).reshape(256, H, 128)
    elbe = (np.asarray(inputs["enemy_lin_b"], f32)
            + np.einsum("o,ohj->j", np.asarray(inputs["enemy_conv_b"], f32),
                        elw3, optimize=True))
    GE = _build_G(inputs["enemy_emb"], inputs["enemy_conv_w"],
                  inputs["enemy_lin_w"], los, his)
    GE += elbe[None, None, :] / H
    se = 240.0 / float(np.abs(GE).max())
    GEq = np.ascontiguousarray((GE * se).reshape(NSYM, H * 128)).astype(fp8)

    flw3 = np.asarray(inputs["friend_lin1_w"], f32).reshape(256, H, 128)
    f2w = np.asarray(inputs["friend_lin2_w"], f32)
    flbe = (np.asarray(inputs["friend_lin1_b"], f32)
            + np.einsum("o,ohj->j", np.asarray(inputs["friend_conv_b"], f32),
                        flw3, optimize=True))
    f2be = flbe @ f2w + np.asarray(inputs["friend_lin2_b"], f32)
    GF = _build_G(inputs["friend_emb"], inputs["friend_conv_w"],
                  inputs["friend_lin1_w"], los, his, out_w=f2w)
    GF += f2be[None, None, :] / H
    GFq = np.ascontiguousarray(GF.reshape(NSYM, H * 14)).astype(bf16)

    mcw = np.asarray(inputs["manip_conv_w"], f32)[:, :, :, 1]  # [64,128,3]
    s_int = mcw.sum(2)
    s12 = mcw[:, :, 1] + mcw[:, :, 2]
    s01 = mcw[:, :, 0] + mcw[:, :, 1]
    wsumT = np.concatenate([s_int.T, s12.T, s01.T], axis=1).astype(f32)
    mlw3 = np.asarray(inputs["manip_lin_w"], f32).reshape(64, 128, 256)
    mlwS = np.concatenate([mlw3[:, 1:127].sum(1), mlw3[:, 0], mlw3[:, 127]],
                          axis=1).astype(f32)

    common = {
        "GE": GEq, "GF": GFq,
        "wsumT": np.ascontiguousarray(wsumT),
        "mlwS": np.ascontiguousarray(mlwS),
        "mcb": np.ascontiguousarray(inputs["manip_conv_b"], f32),
        "mlb": np.ascontiguousarray(inputs["manip_lin_b"], f32),
    }
    x = np.asarray(inputs["x"], np.int64)
    xlo = np.minimum(x[:, 0::2], x[:, 1::2])
    xhi = np.maximum(x[:, 0::2], x[:, 1::2])
    pid = xhi + xlo * (27 - xlo) // 2                  # [B, 128] ints < 105
    maps = []
    for c in range(NCORES):
        rowE = pid[c * BC:(c + 1) * BC].T.reshape(BC * H)   # h-major flat
        ohe = np.ascontiguousarray(
            (np.arange(NSYM)[:, None] == rowE[None, :]).astype(fp8))
        maps.append(dict(common, OHE=ohe))
    return maps, 1.0 / se


def kernel(**inputs):
    in_maps, inv_se = prep_inputs(inputs)
    nc = _get_nc(inv_se)
    res = run_bass_kernel_spmd(nc, in_maps, core_ids=list(range(NCORES)))
    return np.concatenate([r["out"] for r in res.results], axis=0)


# revision 12
# speedup vs baseline: 2.0739x; 1.0217x over previous
"""Trainium2 Bass kernel for nn_Network_67388036874689.

Data-parallel over batch: B=256 as 32 samples on each of 8 cores; params
replicated (host-precomposed).

Structure exploited (validated against the reference on host):
  - fog_of_war's greedy scan returns arange(B) -> permutation is identity.
  - conv2d(3x3, pad=1) on [C, H, 1] only sees kernel column 1 -> 1D 3-tap
    conv over H.
  - THE BIG FUSION: embedding + pair-maxpool + conv + linear collapse into
    a single gathered table per branch:
        logits[b, j] = sum_h G[pid[b, h], h, j]
    with G[p, h, j] = sum_kh CW_kh[p, :] @ W[:, h+1-kh, j] (host-built,
    boundary-clipped), pid = canonical unordered pair index (105 rows,
    pair-max is symmetric).  Linear bias folds in as bias[j]/128 added to
    every G[p, h, j] (exactly one p fires per h).
  - On device each branch is 128 accumulating matmuls
    lpsum[32, J] += OH[:, h]^T @ G[:, h, :], 2-way col-tiled
    (tile_position=(0,32*(h%2))) so two streams share the PE array and the
    final combine is one copy + one add.
  - The ENEMY one-hot depends only on the host-known input x, so it is
    built on the host ([105, 4096] fp8) and streamed interleaved with the
    GE pieces in consumption order -> the enemy phase is pure DMA-paced
    gather with zero DVE work.
  - Enemy table fp8 e4m3 (scaled by sE; descale folded into Exp's scale);
    host sim rel err 3.8e-4 vs 2e-2 tolerance.  Friend table bf16.
  - Manipulator conv input is constant over h -> collapses to 3 reduced
    64x256 matrices (host-summed); f32/f32r path (token discretization is
    precision-sensitive).
  - floor via the 2^23 round-to-nearest magic + is_ge correction; mod 14
    via 2 conditional subtracts; friend pair index = canonical sym index
    from ALU min/max: idx = hi + lo*(27-lo)/2.
  - Friend one-hot: tokens are device-data, so the index row is
    transposed (PE), flattened (SBUF->SBUF DMA), partition-broadcast to
    105 rows per 512-col block (DMA, keeps PE/DVE free), then one
    bf16 is_equal per block.

DMA: sync HWDGE ring carries OHE_k/GE_k interleaved, GF, the idx-row
flatten + broadcasts, and the output; small manipulator weights on the
gpsimd SWDGE ring.
"""

import numpy as np
import ml_dtypes
from contextlib import ExitStack

import concourse.bass as bass
import concourse.bacc as bacc
import concourse.mybir as mybir
import concourse.tile as tile
from concourse.masks import make_identity
from concourse.bass_utils import run_bass_kernel_spmd

F32 = mybir.dt.float32
F32R = mybir.dt.float32r
BF16 = mybir.dt.bfloat16
FP8 = mybir.dt.float8e4
I32 = mybir.dt.int32
AF = mybir.ActivationFunctionType
ALU = mybir.AluOpType
AX = mybir.AxisListType

NCORES = 8
B = 256
BC = B // NCORES        # 32 samples per core
L = 256
V = 14
H = L // 2              # 128 pooled positions
NSYM = V * (V + 1) // 2  # 105 canonical pairs
DEBUG_TAPS = False


def _dram_inputs(nc):
    t = {}

    def inp(name, shape, dt):
        t[name] = nc.dram_tensor(name, list(shape), dt, kind="ExternalInput").ap()

    inp("EB", (NSYM, 8 * 2560), FP8)     # per 16-h block: [one-hot 512 | GE 2048]
    inp("GF", (NSYM, H * 14), BF16)      # col = h*14 + n, bias folded
    inp("wsumT", (128, 192), F32R)       # col = v*64 + o
    inp("mlwS", (64, 768), F32R)         # col = v*256 + j
    inp("mcb", (64,), F32)
    inp("mlb", (256,), F32)
    t["out"] = nc.dram_tensor("out", [BC, 14], F32, kind="ExternalOutput").ap()
    return t


def _tap(nc, io, name, ap):
    if not DEBUG_TAPS:
        return
    t = nc.dram_tensor("tap_" + name, list(ap.shape), ap.dtype,
                       kind="ExternalOutput").ap()
    io["tap_" + name] = t
    nc.gpsimd.dma_start(t, ap)


def build_kernel(nc, tc, ctx, inv_se):
    io = _dram_inputs(nc)
    consts = ctx.enter_context(tc.tile_pool(name="consts", bufs=1))
    work = ctx.enter_context(tc.tile_pool(name="work", bufs=1))
    gepool = ctx.enter_context(tc.tile_pool(name="gepool", bufs=8))
    ohfpool = ctx.enter_context(tc.tile_pool(name="ohfpool", bufs=8))
    ppp = ctx.enter_context(tc.tile_pool(name="ppp", bufs=2, space="PSUM"))
    pacc = ctx.enter_context(tc.tile_pool(name="pacc", bufs=1, space="PSUM"))
    psm = ctx.enter_context(tc.tile_pool(name="psm", bufs=1, space="PSUM"))
    cxp = ctx.enter_context(tc.tile_pool(name="cxp", bufs=2, space="PSUM"))

    def ctile(shape, dt, tag):
        return consts.tile(shape, dt, tag=tag, name=tag)

    def wtile(shape, dt, tag):
        return work.tile(shape, dt, tag=tag, name=tag)

    # ---------------- constants & small weights ----------------
    identF = ctile([128, 128], F32, "identF")
    make_identity(nc, identF)
    iota_i = ctile([128, 1], I32, "iota_i")
    nc.gpsimd.iota(iota_i[:, :], pattern=[[0, 1]], base=0, channel_multiplier=1)
    iota_col = ctile([128, 1], F32, "iota_col")
    nc.vector.tensor_copy(iota_col[:, :], iota_i[:, :])

    # merged [one-hot | GE] blocks, alternating across both HWDGE rings
    # (descriptor generation is ~8ns x 105 partitions per DMA; two rings
    # generate in parallel)
    EBp = []
    for k in range(8):
        t = gepool.tile([NSYM, 2560], FP8, tag="gep", name=f"eb{k}")
        eng = nc.sync if k % 2 == 0 else nc.scalar
        eng.dma_start(t[:, :], io["EB"][:, k * 2560:(k + 1) * 2560])
        EBp.append(t)
    GF = ctile([NSYM, H * 14], BF16, "GF")
    nc.scalar.dma_start(GF[:, :], io["GF"])

    # small manipulator weights on the gpsimd ring
    mcb_col = ctile([64, 1], F32, "mcb")
    nc.gpsimd.dma_start(mcb_col[:, :], io["mcb"])
    mlbB = ctile([BC, 256], F32, "mlbB")
    nc.gpsimd.dma_start(mlbB[:, :], io["mlb"][None, :].partition_broadcast(BC))
    wsumT = ctile([128, 192], F32R, "wsumT")
    nc.gpsimd.dma_start(wsumT[:, :], io["wsumT"])
    mlwS = ctile([64, 768], F32R, "mlwS")
    nc.gpsimd.dma_start(mlwS[:, :], io["mlwS"])

    # ---------------- enemy branch: pure DMA-paced gather ----------------
    lpE = pacc.tile([64, 128], F32, tag="lp", name="lpE")
    for k in range(8):
        for hh in range(16):
            h = k * 16 + hh
            j = h % 2
            nc.tensor.matmul(
                lpE[32 * j:32 * (j + 1), :],
                EBp[k][:, hh * 32:(hh + 1) * 32],
                EBp[k][:, 512 + hh * 128:512 + (hh + 1) * 128],
                start=(h < 2), stop=(h >= 126),
                tile_position=(0, 32 * j), skip_group_check=True)

    cE1 = wtile([BC, 128], F32, "cE1")
    nc.vector.tensor_copy(cE1[:, :], lpE[32:64, :])
    logitsE = wtile([BC, 128], F32, "logitsE")
    nc.vector.tensor_tensor(logitsE[:, :], lpE[0:32, :], cE1[:, :], ALU.add)
    _tap(nc, io, "logitsE", logitsE[:, :])
    # softmax (descale 1/sE folded into Exp scale; sum fused via accum_out)
    ExE = wtile([BC, 128], F32, "ExE")
    smE = wtile([BC, 1], F32, "smE")
    nc.scalar.activation(ExE[:, :], logitsE[:, :], AF.Exp, scale=float(inv_se),
                         accum_out=smE[:, :])
    rsE = wtile([BC, 1], F32, "rsE")
    nc.vector.reciprocal(rsE[:, :], smE[:, :])
    eout = wtile([BC, 128], F32, "eout")
    nc.vector.tensor_scalar(eout[:, :], ExE[:, :], rsE[:, :], None, ALU.mult)
    _tap(nc, io, "eout", eout[:, :])

    # ---------------- manipulator ----------------
    tpv = psm.tile([128, BC], F32, tag="tp", name="tpv")
    nc.tensor.transpose(tpv[:, :], eout[:, :], identF[0:BC, 0:BC])
    vT = wtile([128, BC], F32R, "vT")
    nc.vector.tensor_copy(vT[:, :], tpv[:, :])
    cxs = {}
    for i, v in enumerate(("int", "h0", "hL")):
        cx = cxp.tile([64, BC], F32, tag="cx", name=f"cx{v}")
        nc.tensor.matmul(cx[:, :], wsumT[:, i * 64:(i + 1) * 64], vT[:, :],
                         start=True, stop=True)
        cxs[v] = wtile([64, BC], F32R, f"cxs_{v}")
        nc.scalar.activation(cxs[v][:, :], cx[:, :], AF.Relu, bias=mcb_col[:, :])
    mp = psm.tile([BC, 256], F32, tag="mp", name="mp")
    for i, v in enumerate(("int", "h0", "hL")):
        nc.tensor.matmul(mp[:, :], cxs[v][:, :], mlwS[:, i * 256:(i + 1) * 256],
                         start=(i == 0), stop=(i == 2))
    m_sb = wtile([BC, 256], F32, "m_sb")
    nc.vector.tensor_tensor(m_sb[:, :], mp[:, :], mlbB[:, :], ALU.add)
    _tap(nc, io, "m", m_sb[:, :])

    # tokens = floor(|m|*100) mod 14.  floor(t) = round(t + (2^23-0.5)) - 2^23
    # (round-down magic; t<0.25 yields -0.5, fixed by the max-clamp), then
    # hardware mod.
    tt = wtile([BC, 256], F32, "tt")
    nc.scalar.activation(tt[:, :], m_sb[:, :], AF.Abs, scale=100.0)
    fr = wtile([BC, 256], F32, "fr")
    nc.vector.tensor_scalar(fr[:, :], tt[:, :], 8388607.5, 8388608.0,
                            ALU.add, ALU.subtract)
    fc = wtile([BC, 256], F32, "fc")
    nc.vector.tensor_scalar(fc[:, :], fr[:, :], 0.0, None, ALU.max)
    ti = wtile([BC, 256], F32, "ti")
    nc.vector.tensor_scalar(ti[:, :], fc[:, :], float(V), None, ALU.is_ge)
    t1 = wtile([BC, 256], F32, "t1")
    nc.vector.scalar_tensor_tensor(t1[:, :], ti[:, :], -float(V), fc[:, :],
                                   ALU.mult, ALU.add)
    t2 = wtile([BC, 256], F32, "t2")
    nc.vector.tensor_scalar(t2[:, :], t1[:, :], float(V), None, ALU.is_ge)
    tok = wtile([BC, 256], F32, "tok")
    nc.vector.scalar_tensor_tensor(tok[:, :], t2[:, :], -float(V), t1[:, :],
                                   ALU.mult, ALU.add)
    _tap(nc, io, "tok", tok[:, :])

    # canonical sym pair index via ALU min/max:
    # idx = hi + lo*(27-lo)/2 = hi + 13.5*lo - 0.5*lo^2
    e_, o_ = tok[:, 0:256:2], tok[:, 1:256:2]
    lo_ = wtile([BC, H], F32, "lo_")
    nc.vector.tensor_tensor(lo_[:, :], e_, o_, ALU.min)
    hi_ = wtile([BC, H], F32, "hi_")
    nc.vector.tensor_tensor(hi_[:, :], e_, o_, ALU.max)
    q_ = wtile([BC, H], F32, "q_")
    nc.vector.tensor_tensor(q_[:, :], lo_[:, :], lo_[:, :], ALU.mult)
    u1 = wtile([BC, H], F32, "u1")
    nc.vector.scalar_tensor_tensor(u1[:, :], lo_[:, :], 13.5, hi_[:, :],
                                   ALU.mult, ALU.add)
    idxF = wtile([BC, H], F32, "idxF")
    nc.vector.scalar_tensor_tensor(idxF[:, :], q_[:, :], -0.5, u1[:, :],
                                   ALU.mult, ALU.add)
    _tap(nc, io, "idxF", idxF[:, :])

    # transpose to h-major, flatten to a single row (2 chunk DMAs)
    tpF = psm.tile([128, BC], F32, tag="tp", name="tpF")
    nc.tensor.transpose(tpF[:, :], idxF[:, :], identF[0:BC, 0:BC])
    idxFT = wtile([128, BC], BF16, "idxFT")
    nc.vector.tensor_copy(idxFT[:, :], tpF[:, :])
    idxrowF = wtile([1, BC * H], BF16, "idxrowF")
    for c, eng in enumerate((nc.sync, nc.scalar)):
        eng.dma_start(idxrowF[:, c * 2048:(c + 1) * 2048],
                      idxFT[c * 64:(c + 1) * 64, :])

    # ---------------- friend branch ----------------
    ones_row = ctile([1, NSYM], BF16, "ones_row")
    nc.vector.memset(ones_row[:, :], 1.0)
    lpF = pacc.tile([128, 14], F32, tag="lp", name="lpF")
    for k in range(8):
        pp = ppp.tile([NSYM, 512], F32, tag="pp", name="pp")
        nc.tensor.matmul(pp[:, :], ones_row[:, :],
                         idxrowF[:, k * 512:(k + 1) * 512],
                         start=True, stop=True)
        ohf = ohfpool.tile([NSYM, 512], BF16, tag="ohf", name=f"ohf{k}")
        nc.vector.tensor_scalar(ohf[:, :], pp[:, :], iota_col[0:NSYM, :],
                                None, ALU.is_equal)
        for hh in range(16):
            h = k * 16 + hh
            j = h % 4
            nc.tensor.matmul(
                lpF[32 * j:32 * (j + 1), 0:14],
                ohf[:, hh * 32:(hh + 1) * 32],
                GF[:, h * 14:(h + 1) * 14],
                start=(h < 4), stop=(h >= 124),
                tile_position=(0, 32 * j), skip_group_check=True)

    cF1 = wtile([BC, 14], F32, "cF1")
    nc.vector.tensor_copy(cF1[:, :], lpF[32:64, 0:14])
    cF2 = wtile([BC, 14], F32, "cF2")
    nc.vector.tensor_copy(cF2[:, :], lpF[64:96, 0:14])
    cF3 = wtile([BC, 14], F32, "cF3")
    nc.vector.tensor_copy(cF3[:, :], lpF[96:128, 0:14])
    b01 = wtile([BC, 14], F32, "b01")
    nc.vector.tensor_tensor(b01[:, :], lpF[0:32, 0:14], cF1[:, :], ALU.add)
    b23 = wtile([BC, 14], F32, "b23")
    nc.vector.tensor_tensor(b23[:, :], cF2[:, :], cF3[:, :], ALU.add)
    logitsF = wtile([BC, 14], F32, "logitsF")
    nc.vector.tensor_tensor(logitsF[:, :], b01[:, :], b23[:, :], ALU.add)
    ex = wtile([BC, 14], F32, "ex")
    sm = wtile([BC, 1], F32, "sm")
    nc.scalar.activation(ex[:, :], logitsF[:, :], AF.Exp, accum_out=sm[:, :])
    rs = wtile([BC, 1], F32, "rs")
    nc.vector.reciprocal(rs[:, :], sm[:, :])
    outt = wtile([BC, 14], F32, "outt")
    nc.vector.tensor_scalar(outt[:, :], ex[:, :], rs[:, :], None, ALU.mult)
    nc.sync.dma_start(io["out"], outt[:, :])


_CACHE = {}


def _get_nc(inv_se):
    key = ("nc", round(float(inv_se), 10))
    if key not in _CACHE:
        nc = bacc.Bacc("TRN2", target_bir_lowering=False, debug=False,
                       num_devices=NCORES)
        with tile.TileContext(nc) as tc:
            with ExitStack() as ctx:
                build_kernel(nc, tc, ctx, inv_se)
        nc.compile()
        _CACHE[key] = nc
    return _CACHE[key]


def _build_G(emb, conv_w, lin_w, t0, t1, out_w=None):
    """G[p, h, j]: logits[b, j] = sum_h G[pid[b, h], h, j]."""
    f32 = np.float32
    emb = np.asarray(emb, f32)
    cw = np.ascontiguousarray(np.asarray(conv_w, f32)[:, :, :, 1])  # [O,I,3]
    table = np.maximum(emb[t0], emb[t1])                            # [P,512]
    CW = [table @ cw[:, :, kh].T for kh in range(3)]                # [P,256]
    W = np.asarray(lin_w, f32).reshape(256, H, -1)                  # [O,H,J]
    if out_w is not None:
        W = np.einsum("ohj,jn->ohn", W, np.asarray(out_w, f32))
    G = np.einsum("po,ohj->phj", CW[1], W)
    G[:, 0:H - 1] += np.einsum("po,ohj->phj", CW[0], W[:, 1:H])
    G[:, 1:H] += np.einsum("po,ohj->phj", CW[2], W[:, 0:H - 1])
    return G


def prep_inputs(inputs):
    """Host-side composition + shard. Returns (list of 8 in_maps, inv_se)."""
    f32 = np.float32
    bf16 = ml_dtypes.bfloat16
    fp8 = ml_dtypes.float8_e4m3fn

    los, his = zip(*[(lo, hi) for lo in range(V) for hi in range(lo, V)])
    los, his = np.array(los), np.array(his)

    elw3 = np.asarray(inputs["enemy_lin_w"], f32).reshape(256, H, 128)
    elbe = (np.asarray(inputs["enemy_lin_b"], f32)
            + np.einsum("o,ohj->j", np.asarray(inputs["enemy_conv_b"], f32),
                        elw3, optimize=True))
    GE = _build_G(inputs["enemy_emb"], inputs["enemy_conv_w"],
                  inputs["enemy_lin_w"], los, his)
    GE += elbe[None, None, :] / H
    se = 240.0 / float(np.abs(GE).max())
    GEq = np.ascontiguousarray((GE * se).reshape(NSYM, H * 128)).astype(fp8)

    flw3 = np.asarray(inputs["friend_lin1_w"], f32).reshape(256, H, 128)
    f2w = np.asarray(inputs["friend_lin2_w"], f32)
    flbe = (np.asarray(inputs["friend_lin1_b"], f32)
            + np.einsum("o,ohj->j", np.asarray(inputs["friend_conv_b"], f32),
                        flw3, optimize=True))
    f2be = flbe @ f2w + np.asarray(inputs["friend_lin2_b"], f32)
    GF = _build_G(inputs["friend_emb"], inputs["friend_conv_w"],
                  inputs["friend_lin1_w"], los, his, out_w=f2w)
    GF += f2be[None, None, :] / H
    GFq = np.ascontiguousarray(GF.reshape(NSYM, H * 14)).astype(bf16)

    mcw = np.asarray(inputs["manip_conv_w"], f32)[:, :, :, 1]  # [64,128,3]
    s_int = mcw.sum(2)
    s12 = mcw[:, :, 1] + mcw[:, :, 2]
    s01 = mcw[:, :, 0] + mcw[:, :, 1]
    wsumT = np.concatenate([s_int.T, s12.T, s01.T], axis=1).astype(f32)
    mlw3 = np.asarray(inputs["manip_lin_w"], f32).reshape(64, 128, 256)
    mlwS = np.concatenate([mlw3[:, 1:127].sum(1), mlw3[:, 0], mlw3[:, 127]],
                          axis=1).astype(f32)

    common = {
        "GF": GFq,
        "wsumT": np.ascontiguousarray(wsumT),
        "mlwS": np.ascontiguousarray(mlwS),
        "mcb": np.ascontiguousarray(inputs["manip_conv_b"], f32),
        "mlb": np.ascontiguousarray(inputs["manip_lin_b"], f32),
    }
    x = np.asarray(inputs["x"], np.int64)
    xlo = np.minimum(x[:, 0::2], x[:, 1::2])
    xhi = np.maximum(x[:, 0::2], x[:, 1::2])
    pid = xhi + xlo * (27 - xlo) // 2                  # [B, 128] ints < 105
    maps = []
    for c in range(NCORES):
        rowE = pid[c * BC:(c + 1) * BC].T.reshape(BC * H)   # h-major flat
        ohe = (np.arange(NSYM)[:, None] == rowE[None, :]).astype(fp8)
        eb = np.empty((NSYM, 8 * 2560), fp8)
        for k in range(8):
            eb[:, k * 2560:k * 2560 + 512] = ohe[:, k * 512:(k + 1) * 512]
            eb[:, k * 2560 + 512:(k + 1) * 2560] = GEq[:, k * 2048:(k + 1) * 2048]
        maps.append(dict(common, EB=np.ascontiguousarray(eb)))
    return maps, 1.0 / se


def kernel(**inputs):
    in_maps, inv_se = prep_inputs(inputs)
    nc = _get_nc(inv_se)
    res = run_bass_kernel_spmd(nc, in_maps, core_ids=list(range(NCORES)))
    return np.concatenate([r["out"] for r in res.results], axis=0)


# revision 13
# speedup vs baseline: 2.1340x; 1.0290x over previous
"""Trainium2 Bass kernel for nn_Network_67388036874689.

Data-parallel over batch: B=256 as 32 samples on each of 8 cores; params
replicated (host-precomposed).

Structure exploited (validated against the reference on host):
  - fog_of_war's greedy scan returns arange(B) -> permutation is identity.
  - conv2d(3x3, pad=1) on [C, H, 1] only sees kernel column 1 -> 1D 3-tap
    conv over H.
  - THE BIG FUSION: embedding + pair-maxpool + conv + linear collapse into
    a single gathered table per branch:
        logits[b, j] = sum_h G[pid[b, h], h, j]
    with G[p, h, j] = sum_kh CW_kh[p, :] @ W[:, h+1-kh, j] (host-built,
    boundary-clipped), pid = canonical unordered pair index (105 rows,
    pair-max is symmetric).  Linear bias folds in as bias[j]/128 added to
    every G[p, h, j] (exactly one p fires per h).
  - On device each branch is 128 accumulating matmuls
    lpsum[32, J] += OH[:, h]^T @ G[:, h, :], 2-way col-tiled
    (tile_position=(0,32*(h%2))) so two streams share the PE array and the
    final combine is one copy + one add.
  - The ENEMY one-hot depends only on the host-known input x, so it is
    built on the host ([105, 4096] fp8) and streamed interleaved with the
    GE pieces in consumption order -> the enemy phase is pure DMA-paced
    gather with zero DVE work.
  - Enemy table fp8 e4m3 (scaled by sE; descale folded into Exp's scale);
    host sim rel err 3.8e-4 vs 2e-2 tolerance.  Friend table bf16.
  - Manipulator conv input is constant over h -> collapses to 3 reduced
    64x256 matrices (host-summed); f32/f32r path (token discretization is
    precision-sensitive).
  - floor via the 2^23 round-to-nearest magic + is_ge correction; mod 14
    via 2 conditional subtracts; friend pair index = canonical sym index
    from ALU min/max: idx = hi + lo*(27-lo)/2.
  - Friend one-hot: tokens are device-data, so the index row is
    transposed (PE), flattened (SBUF->SBUF DMA), partition-broadcast to
    105 rows per 512-col block (DMA, keeps PE/DVE free), then one
    bf16 is_equal per block.

DMA: sync HWDGE ring carries OHE_k/GE_k interleaved, GF, the idx-row
flatten + broadcasts, and the output; small manipulator weights on the
gpsimd SWDGE ring.
"""

import numpy as np
import ml_dtypes
from contextlib import ExitStack

import concourse.bass as bass
import concourse.bacc as bacc
import concourse.mybir as mybir
import concourse.tile as tile
from concourse.masks import make_identity
from concourse.bass_utils import run_bass_kernel_spmd

F32 = mybir.dt.float32
F32R = mybir.dt.float32r
BF16 = mybir.dt.bfloat16
FP8 = mybir.dt.float8e4
I32 = mybir.dt.int32
AF = mybir.ActivationFunctionType
ALU = mybir.AluOpType
AX = mybir.AxisListType

NCORES = 8
B = 256
BC = B // NCORES        # 32 samples per core
L = 256
V = 14
H = L // 2              # 128 pooled positions
NSYM = V * (V + 1) // 2  # 105 canonical pairs
DEBUG_TAPS = False


def _dram_inputs(nc):
    t = {}

    def inp(name, shape, dt):
        t[name] = nc.dram_tensor(name, list(shape), dt, kind="ExternalInput").ap()

    inp("EB", (8 * NSYM, 2560), FP8)     # block-major: block k = rows
                                         # [k*105, (k+1)*105), cols
                                         # [one-hot 512 | GE 2048]; each
                                         # block DMA is contiguous in HBM
    inp("GF", (NSYM, H * 14), BF16)      # col = h*14 + n, bias folded
    inp("wsumT", (128, 192), F32R)       # col = v*64 + o
    inp("mlwS", (64, 768), F32R)         # col = v*256 + j
    inp("mcb", (64,), F32)
    inp("mlb", (256,), F32)
    t["out"] = nc.dram_tensor("out", [BC, 14], F32, kind="ExternalOutput").ap()
    return t


def _tap(nc, io, name, ap):
    if not DEBUG_TAPS:
        return
    t = nc.dram_tensor("tap_" + name, list(ap.shape), ap.dtype,
                       kind="ExternalOutput").ap()
    io["tap_" + name] = t
    nc.gpsimd.dma_start(t, ap)


def build_kernel(nc, tc, ctx, inv_se):
    io = _dram_inputs(nc)
    consts = ctx.enter_context(tc.tile_pool(name="consts", bufs=1))
    work = ctx.enter_context(tc.tile_pool(name="work", bufs=1))
    gepool = ctx.enter_context(tc.tile_pool(name="gepool", bufs=8))
    ohfpool = ctx.enter_context(tc.tile_pool(name="ohfpool", bufs=8))
    ppp = ctx.enter_context(tc.tile_pool(name="ppp", bufs=2, space="PSUM"))
    pacc = ctx.enter_context(tc.tile_pool(name="pacc", bufs=1, space="PSUM"))
    psm = ctx.enter_context(tc.tile_pool(name="psm", bufs=1, space="PSUM"))
    cxp = ctx.enter_context(tc.tile_pool(name="cxp", bufs=2, space="PSUM"))

    def ctile(shape, dt, tag):
        return consts.tile(shape, dt, tag=tag, name=tag)

    def wtile(shape, dt, tag):
        return work.tile(shape, dt, tag=tag, name=tag)

    # ---------------- constants & small weights ----------------
    identF = ctile([128, 128], F32, "identF")
    make_identity(nc, identF)
    iota_i = ctile([128, 1], I32, "iota_i")
    nc.gpsimd.iota(iota_i[:, :], pattern=[[0, 1]], base=0, channel_multiplier=1)
    iota_col = ctile([128, 1], F32, "iota_col")
    nc.vector.tensor_copy(iota_col[:, :], iota_i[:, :])

    # merged [one-hot | GE] blocks, block-major in HBM: one contiguous
    # 269KB read per block, in consumption order on the sync ring only
    # (single-ring FIFO keeps per-block completion prompt)
    EBp = []
    for k in range(8):
        t = gepool.tile([NSYM, 2560], FP8, tag="gep", name=f"eb{k}")
        nc.sync.dma_start(t[:, :], io["EB"][k * NSYM:(k + 1) * NSYM, :])
        EBp.append(t)
    GF = ctile([NSYM, H * 14], BF16, "GF")
    nc.scalar.dma_start(GF[:, :], io["GF"])

    # small manipulator weights on the gpsimd ring
    mcb_col = ctile([64, 1], F32, "mcb")
    nc.gpsimd.dma_start(mcb_col[:, :], io["mcb"])
    mlbB = ctile([BC, 256], F32, "mlbB")
    nc.gpsimd.dma_start(mlbB[:, :], io["mlb"][None, :].partition_broadcast(BC))
    wsumT = ctile([128, 192], F32R, "wsumT")
    nc.gpsimd.dma_start(wsumT[:, :], io["wsumT"])
    mlwS = ctile([64, 768], F32R, "mlwS")
    nc.gpsimd.dma_start(mlwS[:, :], io["mlwS"])

    # ---------------- enemy branch: pure DMA-paced gather ----------------
    lpE = pacc.tile([64, 128], F32, tag="lp", name="lpE")
    for k in range(8):
        for hh in range(16):
            h = k * 16 + hh
            j = h % 2
            nc.tensor.matmul(
                lpE[32 * j:32 * (j + 1), :],
                EBp[k][:, hh * 32:(hh + 1) * 32],
                EBp[k][:, 512 + hh * 128:512 + (hh + 1) * 128],
                start=(h < 2), stop=(h >= 126),
                tile_position=(0, 32 * j), skip_group_check=True)

    cE1 = wtile([BC, 128], F32, "cE1")
    nc.vector.tensor_copy(cE1[:, :], lpE[32:64, :])
    logitsE = wtile([BC, 128], F32, "logitsE")
    nc.vector.tensor_tensor(logitsE[:, :], lpE[0:32, :], cE1[:, :], ALU.add)
    _tap(nc, io, "logitsE", logitsE[:, :])
    # softmax (descale 1/sE folded into Exp scale; sum fused via accum_out)
    ExE = wtile([BC, 128], F32, "ExE")
    smE = wtile([BC, 1], F32, "smE")
    nc.scalar.activation(ExE[:, :], logitsE[:, :], AF.Exp, scale=float(inv_se),
                         accum_out=smE[:, :])
    rsE = wtile([BC, 1], F32, "rsE")
    nc.vector.reciprocal(rsE[:, :], smE[:, :])
    eout = wtile([BC, 128], F32, "eout")
    nc.vector.tensor_scalar(eout[:, :], ExE[:, :], rsE[:, :], None, ALU.mult)
    _tap(nc, io, "eout", eout[:, :])

    # ---------------- manipulator ----------------
    tpv = psm.tile([128, BC], F32, tag="tp", name="tpv")
    nc.tensor.transpose(tpv[:, :], eout[:, :], identF[0:BC, 0:BC])
    vT = wtile([128, BC], F32R, "vT")
    nc.vector.tensor_copy(vT[:, :], tpv[:, :])
    cxs = {}
    for i, v in enumerate(("int", "h0", "hL")):
        cx = cxp.tile([64, BC], F32, tag="cx", name=f"cx{v}")
        nc.tensor.matmul(cx[:, :], wsumT[:, i * 64:(i + 1) * 64], vT[:, :],
                         start=True, stop=True)
        cxs[v] = wtile([64, BC], F32R, f"cxs_{v}")
        nc.scalar.activation(cxs[v][:, :], cx[:, :], AF.Relu, bias=mcb_col[:, :])
    mp = psm.tile([BC, 256], F32, tag="mp", name="mp")
    for i, v in enumerate(("int", "h0", "hL")):
        nc.tensor.matmul(mp[:, :], cxs[v][:, :], mlwS[:, i * 256:(i + 1) * 256],
                         start=(i == 0), stop=(i == 2))
    m_sb = wtile([BC, 256], F32, "m_sb")
    nc.vector.tensor_tensor(m_sb[:, :], mp[:, :], mlbB[:, :], ALU.add)
    _tap(nc, io, "m", m_sb[:, :])

    # tokens = floor(|m|*100) mod 14.  floor(t) = round(t + (2^23-0.5)) - 2^23
    # (round-down magic; t<0.25 yields -0.5, fixed by the max-clamp), then
    # hardware mod.
    tt = wtile([BC, 256], F32, "tt")
    nc.scalar.activation(tt[:, :], m_sb[:, :], AF.Abs, scale=100.0)
    fr = wtile([BC, 256], F32, "fr")
    nc.vector.tensor_scalar(fr[:, :], tt[:, :], 8388607.5, 8388608.0,
                            ALU.add, ALU.subtract)
    fc = wtile([BC, 256], F32, "fc")
    nc.vector.tensor_scalar(fc[:, :], fr[:, :], 0.0, None, ALU.max)
    ti = wtile([BC, 256], F32, "ti")
    nc.vector.tensor_scalar(ti[:, :], fc[:, :], float(V), None, ALU.is_ge)
    t1 = wtile([BC, 256], F32, "t1")
    nc.vector.scalar_tensor_tensor(t1[:, :], ti[:, :], -float(V), fc[:, :],
                                   ALU.mult, ALU.add)
    t2 = wtile([BC, 256], F32, "t2")
    nc.vector.tensor_scalar(t2[:, :], t1[:, :], float(V), None, ALU.is_ge)
    tok = wtile([BC, 256], F32, "tok")
    nc.vector.scalar_tensor_tensor(tok[:, :], t2[:, :], -float(V), t1[:, :],
                                   ALU.mult, ALU.add)
    _tap(nc, io, "tok", tok[:, :])

    # canonical sym pair index via ALU min/max:
    # idx = hi + lo*(27-lo)/2 = hi + 13.5*lo - 0.5*lo^2
    e_, o_ = tok[:, 0:256:2], tok[:, 1:256:2]
    lo_ = wtile([BC, H], F32, "lo_")
    nc.vector.tensor_tensor(lo_[:, :], e_, o_, ALU.min)
    hi_ = wtile([BC, H], F32, "hi_")
    nc.vector.tensor_tensor(hi_[:, :], e_, o_, ALU.max)
    q_ = wtile([BC, H], F32, "q_")
    nc.vector.tensor_tensor(q_[:, :], lo_[:, :], lo_[:, :], ALU.mult)
    u1 = wtile([BC, H], F32, "u1")
    nc.vector.scalar_tensor_tensor(u1[:, :], lo_[:, :], 13.5, hi_[:, :],
                                   ALU.mult, ALU.add)
    idxF = wtile([BC, H], F32, "idxF")
    nc.vector.scalar_tensor_tensor(idxF[:, :], q_[:, :], -0.5, u1[:, :],
                                   ALU.mult, ALU.add)
    _tap(nc, io, "idxF", idxF[:, :])

    # transpose to h-major, flatten to a single row (2 chunk DMAs)
    tpF = psm.tile([128, BC], F32, tag="tp", name="tpF")
    nc.tensor.transpose(tpF[:, :], idxF[:, :], identF[0:BC, 0:BC])
    idxFT = wtile([128, BC], BF16, "idxFT")
    nc.vector.tensor_copy(idxFT[:, :], tpF[:, :])
    idxrowF = wtile([1, BC * H], BF16, "idxrowF")
    for c, eng in enumerate((nc.sync, nc.scalar)):
        eng.dma_start(idxrowF[:, c * 2048:(c + 1) * 2048],
                      idxFT[c * 64:(c + 1) * 64, :])

    # ---------------- friend branch ----------------
    ones_row = ctile([1, NSYM], BF16, "ones_row")
    nc.vector.memset(ones_row[:, :], 1.0)
    lpF = pacc.tile([128, 14], F32, tag="lp", name="lpF")
    for k in range(8):
        pp = ppp.tile([NSYM, 512], F32, tag="pp", name="pp")
        nc.tensor.matmul(pp[:, :], ones_row[:, :],
                         idxrowF[:, k * 512:(k + 1) * 512],
                         start=True, stop=True)
        ohf = ohfpool.tile([NSYM, 512], BF16, tag="ohf", name=f"ohf{k}")
        nc.vector.tensor_scalar(ohf[:, :], pp[:, :], iota_col[0:NSYM, :],
                                None, ALU.is_equal)
        for hh in range(16):
            h = k * 16 + hh
            j = h % 4
            nc.tensor.matmul(
                lpF[32 * j:32 * (j + 1), 0:14],
                ohf[:, hh * 32:(hh + 1) * 32],
                GF[:, h * 14:(h + 1) * 14],
                start=(h < 4), stop=(h >= 124),
                tile_position=(0, 32 * j), skip_group_check=True)

    cF1 = wtile([BC, 14], F32, "cF1")
    nc.vector.tensor_copy(cF1[:, :], lpF[32:64, 0:14])
    cF2 = wtile([BC, 14], F32, "cF2")
    nc.vector.tensor_copy(cF2[:, :], lpF[64:96, 0:14])
    cF3 = wtile([BC, 14], F32, "cF3")
    nc.vector.tensor_copy(cF3[:, :], lpF[96:128, 0:14])
    b01 = wtile([BC, 14], F32, "b01")
    nc.vector.tensor_tensor(b01[:, :], lpF[0:32, 0:14], cF1[:, :], ALU.add)
    b23 = wtile([BC, 14], F32, "b23")
    nc.vector.tensor_tensor(b23[:, :], cF2[:, :], cF3[:, :], ALU.add)
    logitsF = wtile([BC, 14], F32, "logitsF")
    nc.vector.tensor_tensor(logitsF[:, :], b01[:, :], b23[:, :], ALU.add)
    ex = wtile([BC, 14], F32, "ex")
    sm = wtile([BC, 1], F32, "sm")
    nc.scalar.activation(ex[:, :], logitsF[:, :], AF.Exp, accum_out=sm[:, :])
    rs = wtile([BC, 1], F32, "rs")
    nc.vector.reciprocal(rs[:, :], sm[:, :])
    outt = wtile([BC, 14], F32, "outt")
    nc.vector.tensor_scalar(outt[:, :], ex[:, :], rs[:, :], None, ALU.mult)
    nc.sync.dma_start(io["out"], outt[:, :])


_CACHE = {}


def _get_nc(inv_se):
    key = ("nc", round(float(inv_se), 10))
    if key not in _CACHE:
        nc = bacc.Bacc("TRN2", target_bir_lowering=False, debug=False,
                       num_devices=NCORES)
        with tile.TileContext(nc) as tc:
            with ExitStack() as ctx:
                build_kernel(nc, tc, ctx, inv_se)
        nc.compile()
        _CACHE[key] = nc
    return _CACHE[key]


def _build_G(emb, conv_w, lin_w, t0, t1, out_w=None):
    """G[p, h, j]: logits[b, j] = sum_h G[pid[b, h], h, j]."""
    f32 = np.float32
    emb = np.asarray(emb, f32)
    cw = np.ascontiguousarray(np.asarray(conv_w, f32)[:, :, :, 1])  # [O,I,3]
    table = np.maximum(emb[t0], emb[t1])                            # [P,512]
    CW = [table @ cw[:, :, kh].T for kh in range(3)]                # [P,256]
    W = np.asarray(lin_w, f32).reshape(256, H, -1)                  # [O,H,J]
    if out_w is not None:
        W = np.einsum("ohj,jn->ohn", W, np.asarray(out_w, f32))
    G = np.einsum("po,ohj->phj", CW[1], W)
    G[:, 0:H - 1] += np.einsum("po,ohj->phj", CW[0], W[:, 1:H])
    G[:, 1:H] += np.einsum("po,ohj->phj", CW[2], W[:, 0:H - 1])
    return G


def prep_inputs(inputs):
    """Host-side composition + shard. Returns (list of 8 in_maps, inv_se)."""
    f32 = np.float32
    bf16 = ml_dtypes.bfloat16
    fp8 = ml_dtypes.float8_e4m3fn

    los, his = zip(*[(lo, hi) for lo in range(V) for hi in range(lo, V)])
    los, his = np.array(los), np.array(his)

    elw3 = np.asarray(inputs["enemy_lin_w"], f32).reshape(256, H, 128)
    elbe = (np.asarray(inputs["enemy_lin_b"], f32)
            + np.einsum("o,ohj->j", np.asarray(inputs["enemy_conv_b"], f32),
                        elw3, optimize=True))
    GE = _build_G(inputs["enemy_emb"], inputs["enemy_conv_w"],
                  inputs["enemy_lin_w"], los, his)
    GE += elbe[None, None, :] / H
    se = 240.0 / float(np.abs(GE).max())
    GEq = np.ascontiguousarray((GE * se).reshape(NSYM, H * 128)).astype(fp8)

    flw3 = np.asarray(inputs["friend_lin1_w"], f32).reshape(256, H, 128)
    f2w = np.asarray(inputs["friend_lin2_w"], f32)
    flbe = (np.asarray(inputs["friend_lin1_b"], f32)
            + np.einsum("o,ohj->j", np.asarray(inputs["friend_conv_b"], f32),
                        flw3, optimize=True))
    f2be = flbe @ f2w + np.asarray(inputs["friend_lin2_b"], f32)
    GF = _build_G(inputs["friend_emb"], inputs["friend_conv_w"],
                  inputs["friend_lin1_w"], los, his, out_w=f2w)
    GF += f2be[None, None, :] / H
    GFq = np.ascontiguousarray(GF.reshape(NSYM, H * 14)).astype(bf16)

    mcw = np.asarray(inputs["manip_conv_w"], f32)[:, :, :, 1]  # [64,128,3]
    s_int = mcw.sum(2)
    s12 = mcw[:, :, 1] + mcw[:, :, 2]
    s01 = mcw[:, :, 0] + mcw[:, :, 1]
    wsumT = np.concatenate([s_int.T, s12.T, s01.T], axis=1).astype(f32)
    mlw3 = np.asarray(inputs["manip_lin_w"], f32).reshape(64, 128, 256)
    mlwS = np.concatenate([mlw3[:, 1:127].sum(1), mlw3[:, 0], mlw3[:, 127]],
                          axis=1).astype(f32)

    common = {
        "GF": GFq,
        "wsumT": np.ascontiguousarray(wsumT),
        "mlwS": np.ascontiguousarray(mlwS),
        "mcb": np.ascontiguousarray(inputs["manip_conv_b"], f32),
        "mlb": np.ascontiguousarray(inputs["manip_lin_b"], f32),
    }
    x = np.asarray(inputs["x"], np.int64)
    xlo = np.minimum(x[:, 0::2], x[:, 1::2])
    xhi = np.maximum(x[:, 0::2], x[:, 1::2])
    pid = xhi + xlo * (27 - xlo) // 2                  # [B, 128] ints < 105
    maps = []
    for c in range(NCORES):
        rowE = pid[c * BC:(c + 1) * BC].T.reshape(BC * H)   # h-major flat
        ohe = (np.arange(NSYM)[:, None] == rowE[None, :]).astype(fp8)
        eb = np.empty((8 * NSYM, 2560), fp8)
        for k in range(8):
            eb[k * NSYM:(k + 1) * NSYM, 0:512] = ohe[:, k * 512:(k + 1) * 512]
            eb[k * NSYM:(k + 1) * NSYM, 512:2560] = GEq[:, k * 2048:(k + 1) * 2048]
        maps.append(dict(common, EB=np.ascontiguousarray(eb)))
    return maps, 1.0 / se


def kernel(**inputs):
    in_maps, inv_se = prep_inputs(inputs)
    nc = _get_nc(inv_se)
    res = run_bass_kernel_spmd(nc, in_maps, core_ids=list(range(NCORES)))
    return np.concatenate([r["out"] for r in res.results], axis=0)
